# revision 1
# baseline (speedup 1.0000x reference)
"""MoE kernel for Trainium2 — 8-core expert-parallel + shared-expert 2D shard.

Strategy:
  - Host computes routing (replica of reference math, fp32) ONLY to decide
    data placement: which tokens go to which expert-core (top-2 dispatch).
    The combine weights used in the output math are recomputed ON DEVICE
    from raw inputs (centroid matmul in fp32 + sigmoid/top2/softmax).
  - Core e (e=0..7): routed expert e's MLP over its gathered tokens
    (capacity-padded to C, invalid rows masked to 0 on device), plus a
    (token-quarter x F-half) shard of the 2 shared experts.
  - Matmuls in float32r (full PE rate); routing matmul in exact float32.
  - Host unshard: scatter-add routed partials (unique indices per core),
    sum shared partials; residual x folded in on-device via x_res halves.
"""

import sys

sys.path.insert(0, "/opt/trn_rl_repo")

import numpy as np

D = 1024
F = 4096          # routed expert hidden
FSH = 4096        # shared shard hidden: 2 experts x (4096/2) F-half
E = 8
NS = 2
B, S = 2, 1024
TT = B * S        # 2048 tokens
TQ = TT // 4      # 512-token quarter per shared shard
FCH = 512         # hidden-chunk streamed per iteration

_prog_cache = {}


def _mchunks(n):
    """Split n into moving-dim chunks of 512 + remainder. Chunks must start
    at multiples of 512 so no matmul output crosses a PSUM bank boundary
    (fp32 bank = 512 floats)."""
    out = [512] * (n // 512)
    if n % 512:
        out.append(n % 512)
    return out


def _build(C):
    from contextlib import ExitStack
    from concourse import bacc, bass, tile, mybir

    f32 = mybir.dt.float32
    f32r = mybir.dt.float32r
    AF = mybir.ActivationFunctionType
    ALU = mybir.AluOpType
    AX = mybir.AxisListType

    nc = bacc.Bacc("TRN2", target_bir_lowering=False, debug=False, num_devices=8)

    d_xgT = nc.dram_tensor("xgT", [D, C], f32, kind="ExternalInput").ap()
    d_xqT = nc.dram_tensor("xqT", [D, TQ], f32r, kind="ExternalInput").ap()
    d_xres = nc.dram_tensor("x_res", [TQ, D], f32, kind="ExternalInput").ap()
    d_centT = nc.dram_tensor("centT", [D, E], f32, kind="ExternalInput").ap()
    d_rbias = nc.dram_tensor("rbias", [128, E], f32, kind="ExternalInput").ap()
    d_sel = nc.dram_tensor("sel", [128, E], f32, kind="ExternalInput").ap()
    d_valid = nc.dram_tensor("valid", [128, C // 128], f32, kind="ExternalInput").ap()
    d_ident = nc.dram_tensor("ident", [128, 128], f32, kind="ExternalInput").ap()
    d_wfc = nc.dram_tensor("wfcT", [D, F], f32r, kind="ExternalInput").ap()
    d_wpj = nc.dram_tensor("wprojT", [F, D], f32r, kind="ExternalInput").ap()
    d_wfcs = nc.dram_tensor("wfcshT", [D, FSH], f32r, kind="ExternalInput").ap()
    d_wpjs = nc.dram_tensor("wprojshT", [FSH, D], f32r, kind="ExternalInput").ap()
    d_outr = nc.dram_tensor("out_r", [C, D], f32, kind="ExternalOutput").ap()
    d_outs = nc.dram_tensor("out_sh", [TQ, D], f32, kind="ExternalOutput").ap()

    CJ = C // 128  # routed token tiles
    QJ = TQ // 128  # shared token tiles

    with tile.TileContext(nc) as tc, ExitStack() as ctx:
        const = ctx.enter_context(tc.tile_pool(name="const", bufs=1))
        xpool = ctx.enter_context(tc.tile_pool(name="xpool", bufs=1))
        ypool = ctx.enter_context(tc.tile_pool(name="ypool", bufs=1))
        rpool = ctx.enter_context(tc.tile_pool(name="rpool", bufs=2))
        wpool = ctx.enter_context(tc.tile_pool(name="wpool", bufs=2))
        hpool = ctx.enter_context(tc.tile_pool(name="hpool", bufs=2))

        # ---- resident loads -------------------------------------------------
        centT = const.tile([128, 8, E], f32)
        for di in range(8):
            nc.sync.dma_start(centT[:, di, :], d_centT[di * 128:(di + 1) * 128, :])
        rbias = const.tile([128, E], f32)
        nc.sync.dma_start(rbias[:], d_rbias[:, :])
        sel = const.tile([128, E], f32)
        nc.sync.dma_start(sel[:], d_sel[:, :])
        valid = const.tile([128, CJ], f32)
        nc.sync.dma_start(valid[:], d_valid[:, :])
        ident = const.tile([128, 128], f32)
        nc.sync.dma_start(ident[:], d_ident[:, :])
        zbias = const.tile([128, 1], f32)
        nc.vector.memset(zbias[:], 0.0)

        xgTr = xpool.tile([128, 8, C], f32r)
        xqT = xpool.tile([128, 8, TQ], f32r)
        for di in range(8):
            nc.sync.dma_start(xqT[:, di, :], d_xqT[di * 128:(di + 1) * 128, :])
        xres = xpool.tile([128, QJ, D], f32)
        for j in range(QJ):
            nc.sync.dma_start(xres[:, j, :], d_xres[j * 128:(j + 1) * 128, :])

        y_r = ypool.tile([128, CJ, D], f32)
        y_sh = ypool.tile([128, QJ, D], f32)
        ce = const.tile([128, CJ], f32)

        # ---- routing (fp32, device-side combine weights) --------------------
        with ExitStack() as rctx:
            xgf = rctx.enter_context(tc.tile_pool(name="xgf", bufs=1))
            rps = rctx.enter_context(tc.tile_pool(name="rps", bufs=1, space="PSUM"))
            tps = rctx.enter_context(tc.tile_pool(name="tps", bufs=2, space="PSUM"))
            xgT = xgf.tile([128, 8, C], f32)
            for di in range(8):
                nc.sync.dma_start(xgT[:, di, :], d_xgT[di * 128:(di + 1) * 128, :])
            # f32r-rounded copy for the MLP matmuls
            for di in range(8):
                nc.vector.tensor_copy(xgTr[:, di, :], xgT[:, di, :])
            raw_ps = rps.tile([E, C], f32)
            for k in range(8):
                off = 0
                for w in _mchunks(C):
                    nc.tensor.matmul(
                        raw_ps[:, off:off + w],
                        lhsT=centT[:, k, :],
                        rhs=xgT[:, k, off:off + w],
                        start=(k == 0),
                        stop=(k == 7),
                    )
                    off += w
            raw_sb = rpool.tile([E, C], f32, tag="rawsb")
            nc.vector.tensor_copy(raw_sb[:], raw_ps[:])

            for j in range(CJ):
                t_ps = tps.tile([128, E], f32)
                nc.tensor.transpose(
                    t_ps[:], raw_sb[:, j * 128:(j + 1) * 128], ident[0:E, 0:E]
                )
                raw_t = rpool.tile([128, E], f32, tag="rawt")
                nc.vector.tensor_copy(raw_t[:], t_ps[:])
                braw = rpool.tile([128, E], f32, tag="braw")
                nc.vector.tensor_add(braw[:], raw_t[:], rbias[:])
                scores = rpool.tile([128, E], f32, tag="scores")
                nc.scalar.activation(scores[:], raw_t[:], AF.Sigmoid, bias=zbias[:])
                maxes = rpool.tile([128, 8], f32, tag="maxes")
                nc.vector.max(maxes[:], braw[:])
                mask1 = rpool.tile([128, E], f32, tag="mask1")
                nc.vector.tensor_tensor(
                    mask1[:], braw[:], maxes[:, 0:1].to_broadcast([128, E]), ALU.is_ge
                )
                mask2 = rpool.tile([128, E], f32, tag="mask2")
                nc.vector.tensor_tensor(
                    mask2[:], braw[:], maxes[:, 1:2].to_broadcast([128, E]), ALU.is_ge
                )
                nc.vector.tensor_sub(mask2[:], mask2[:], mask1[:])
                tmp = rpool.tile([128, E], f32, tag="tmp")
                s1 = rpool.tile([128, 1], f32, tag="s1")
                s2 = rpool.tile([128, 1], f32, tag="s2")
                nc.vector.tensor_mul(tmp[:], mask1[:], scores[:])
                nc.vector.tensor_reduce(s1[:], tmp[:], axis=AX.X, op=ALU.add)
                nc.vector.tensor_mul(tmp[:], mask2[:], scores[:])
                nc.vector.tensor_reduce(s2[:], tmp[:], axis=AX.X, op=ALU.add)
                i1 = rpool.tile([128, 1], f32, tag="i1")
                i2 = rpool.tile([128, 1], f32, tag="i2")
                nc.vector.tensor_mul(tmp[:], mask1[:], sel[:])
                nc.vector.tensor_reduce(i1[:], tmp[:], axis=AX.X, op=ALU.add)
                nc.vector.tensor_mul(tmp[:], mask2[:], sel[:])
                nc.vector.tensor_reduce(i2[:], tmp[:], axis=AX.X, op=ALU.add)
                w1 = rpool.tile([128, 1], f32, tag="w1")
                nc.vector.tensor_sub(s1[:], s1[:], s2[:])
                nc.scalar.activation(w1[:], s1[:], AF.Sigmoid, bias=zbias[:])
                w2 = rpool.tile([128, 1], f32, tag="w2")
                nc.vector.tensor_scalar(
                    w2[:], w1[:], -1.0, 1.0, op0=ALU.mult, op1=ALU.add
                )
                nc.vector.tensor_mul(i1[:], i1[:], w1[:])
                nc.vector.tensor_mul(i2[:], i2[:], w2[:])
                nc.vector.tensor_add(i1[:], i1[:], i2[:])
                nc.vector.tensor_mul(ce[:, j:j + 1], i1[:], valid[:, j:j + 1])

        # ---- MLP passes -----------------------------------------------------
        with tc.tile_pool(name="ph", bufs=2, space="PSUM") as php, \
             tc.tile_pool(name="py", bufs=2, space="PSUM") as pyp:

            def mlp(xsb, d_wfc_, d_wpj_, yacc, tcnt, fh):
                njt = tcnt // 128
                nch = fh // FCH
                mt = FCH // 128
                for ci in range(nch):
                    wfc_t = wpool.tile([128, 8, FCH], f32r, tag="wfc")
                    for di in range(8):
                        nc.sync.dma_start(
                            wfc_t[:, di, :],
                            d_wfc_[di * 128:(di + 1) * 128,
                                   ci * FCH:(ci + 1) * FCH],
                        )
                    wpj_t = wpool.tile([128, mt, D], f32r, tag="wpj")
                    for mi in range(mt):
                        nc.sync.dma_start(
                            wpj_t[:, mi, :],
                            d_wpj_[ci * FCH + mi * 128:ci * FCH + (mi + 1) * 128, :],
                        )
                    hs = []
                    for mi in range(mt):
                        ph = php.tile([128, C], f32, tag="ph")
                        for k in range(8):
                            off = 0
                            for w in _mchunks(tcnt):
                                nc.tensor.matmul(
                                    ph[:, off:off + w],
                                    lhsT=wfc_t[:, k, mi * 128:(mi + 1) * 128],
                                    rhs=xsb[:, k, off:off + w],
                                    start=(k == 0),
                                    stop=(k == 7),
                                )
                                off += w
                        h = hpool.tile([128, C], f32r, tag=f"h{mi}")
                        nc.scalar.activation(
                            h[:, 0:tcnt], ph[:, 0:tcnt], AF.Gelu, bias=zbias[:]
                        )
                        hs.append(h)
                    for j in range(njt):
                        py = pyp.tile([128, D], f32, tag="py")
                        for mi in range(mt):
                            off = 0
                            for w in _mchunks(D):
                                nc.tensor.matmul(
                                    py[:, off:off + w],
                                    lhsT=hs[mi][:, j * 128:(j + 1) * 128],
                                    rhs=wpj_t[:, mi, off:off + w],
                                    start=(mi == 0),
                                    stop=(mi == mt - 1),
                                )
                                off += w
                        if ci == 0:
                            nc.vector.tensor_copy(yacc[:, j, :], py[:])
                        else:
                            nc.vector.tensor_add(yacc[:, j, :], yacc[:, j, :], py[:])

            mlp(xgTr, d_wfc, d_wpj, y_r, C, F)
            mlp(xqT, d_wfcs, d_wpjs, y_sh, TQ, FSH)

        # ---- finalize + store ----------------------------------------------
        for j in range(CJ):
            nc.vector.tensor_scalar_mul(y_r[:, j, :], y_r[:, j, :], ce[:, j:j + 1])
            nc.sync.dma_start(d_outr[j * 128:(j + 1) * 128, :], y_r[:, j, :])
        for j in range(QJ):
            nc.vector.tensor_add(y_sh[:, j, :], y_sh[:, j, :], xres[:, j, :])
            nc.sync.dma_start(d_outs[j * 128:(j + 1) * 128, :], y_sh[:, j, :])

    if not nc.is_finalized():
        nc.finalize()
    return nc


def kernel(x, centroids, routing_bias, Wfc_r, Wproj_r, Wfc_sh, Wproj_sh):
    from concourse import bass_utils

    x = np.asarray(x, np.float32)
    centroids = np.asarray(centroids, np.float32)
    routing_bias = np.asarray(routing_bias, np.float32)
    Wfc_r = np.asarray(Wfc_r, np.float32)
    Wproj_r = np.asarray(Wproj_r, np.float32)
    Wfc_sh = np.asarray(Wfc_sh, np.float32)
    Wproj_sh = np.asarray(Wproj_sh, np.float32)

    xf = np.ascontiguousarray(x.reshape(TT, D))

    # host routing — data placement only (device recomputes combine weights)
    raw = xf @ centroids.T
    balanced = raw + routing_bias[None, :]
    top2 = np.argsort(-balanced, axis=-1, kind="stable")[:, :2]
    idx_lists = []
    for e in range(E):
        hit = (top2 == e).any(axis=1)
        idx_lists.append(np.nonzero(hit)[0].astype(np.int64))
    nmax = max(len(ix) for ix in idx_lists)
    C = max(256, ((nmax + 127) // 128) * 128)

    if C not in _prog_cache:
        _prog_cache[C] = _build(C)
    nc = _prog_cache[C]

    xT = np.ascontiguousarray(xf.T)
    ident = np.eye(128, dtype=np.float32)
    rbias128 = np.tile(routing_bias[None, :], (128, 1)).astype(np.float32)
    FH = F // 2  # shared expert F-half

    in_maps = []
    for c in range(E):
        ix = idx_lists[c]
        n = len(ix)
        pad = np.zeros(C, np.int64)
        pad[:n] = ix
        xg = xf[pad]  # [C, D]
        validm = np.zeros((128, C // 128), np.float32)
        for t in range(C):
            if t < n:
                validm[t % 128, t // 128] = 1.0
        sel = np.zeros((128, E), np.float32)
        sel[:, c] = 1.0
        q = c // 2
        half = c % 2
        wfcsh = np.concatenate(
            [Wfc_sh[nn, half * FH:(half + 1) * FH, :] for nn in range(NS)], axis=0
        )  # [FSH, D]
        wpjsh = np.concatenate(
            [Wproj_sh[nn, :, half * FH:(half + 1) * FH] for nn in range(NS)], axis=1
        )  # [D, FSH]
        in_maps.append({
            "xgT": np.ascontiguousarray(xg.T),
            "xqT": np.ascontiguousarray(xT[:, q * TQ:(q + 1) * TQ]),
            "x_res": np.ascontiguousarray(xf[q * TQ:(q + 1) * TQ] * 0.5),
            "centT": np.ascontiguousarray(centroids.T),
            "rbias": rbias128,
            "sel": sel,
            "valid": validm,
            "ident": ident,
            "wfcT": np.ascontiguousarray(Wfc_r[c].T),
            "wprojT": np.ascontiguousarray(Wproj_r[c].T),
            "wfcshT": np.ascontiguousarray(wfcsh.T),
            "wprojshT": np.ascontiguousarray(wpjsh.T),
        })

    globals()["_last_in_maps"] = in_maps
    res = bass_utils.run_bass_kernel_spmd(nc, in_maps, core_ids=list(range(E)))
    globals()["_last_results"] = res

    out = np.zeros((TT, D), np.float32)
    for c in range(E):
        r = res.results[c]
        q = c // 2
        out[q * TQ:(q + 1) * TQ] += r["out_sh"]
        ix = idx_lists[c]
        out[ix] += r["out_r"][:len(ix)]
    return out.reshape(B, S, D)



# revision 2
# speedup vs baseline: 113.2893x; 113.2893x over previous
"""MoE kernel for Trainium2 — 8-core expert-parallel + shared-expert 2D shard.

Strategy:
  - Host computes routing (replica of reference math, fp32) ONLY to decide
    data placement: which tokens go to which expert-core (top-2 dispatch).
    The combine weights used in the output math are recomputed ON DEVICE
    from raw inputs (centroid matmul in fp32 + sigmoid/top2/softmax).
  - Core e (e=0..7): routed expert e's MLP over its gathered tokens
    (capacity-padded to C, invalid rows masked to 0 on device), plus a
    (token-quarter x F-half) shard of the 2 shared experts.
  - MLP matmuls in bf16 (full PE rate, half the HBM traffic); routing in
    exact fp32. PSUM accumulation is always fp32.
  - All DRAM operands use a [128, ntile, free] "partition-tiled" layout so
    each weight chunk is a single >=1MB dma_start.
  - The whole per-iteration body can be wrapped in a hardware For_i loop
    (R iterations); iterations are idempotent, so the R>1 program computes
    the same output while letting a single NEFF dispatch execute the kernel
    R times back-to-back (used to measure true per-execution HW time under
    a high fixed dispatch latency).
  - Host unshard: scatter-add routed partials (unique indices per core),
    sum shared partials; residual x folded in on-device via x_res halves.
"""

import sys

sys.path.insert(0, "/opt/trn_rl_repo")

import numpy as np
import ml_dtypes

BF16 = ml_dtypes.bfloat16

D = 1024
F = 4096          # routed expert hidden
FSH = 4096        # shared shard hidden: 2 experts x (4096/2) F-half
E = 8
NS = 2
B, S = 2, 1024
TT = B * S        # 2048 tokens
TQ = TT // 4      # 512-token quarter per shared shard
FCH = 1024        # hidden-chunk streamed per iteration

_prog_cache = {}


def _mchunks(n):
    """Split n into moving-dim chunks of 512 + remainder. Chunks must start
    at multiples of 512 so no matmul output crosses a PSUM bank boundary
    (fp32 bank = 512 floats)."""
    out = [512] * (n // 512)
    if n % 512:
        out.append(n % 512)
    return out


def _build(C, R=1):
    from contextlib import ExitStack, nullcontext
    from concourse import bacc, bass, tile, mybir

    f32 = mybir.dt.float32
    bf16 = mybir.dt.bfloat16
    AF = mybir.ActivationFunctionType
    ALU = mybir.AluOpType
    AX = mybir.AxisListType
    ET = mybir.EngineType

    nc = bacc.Bacc("TRN2", target_bir_lowering=False, debug=False, num_devices=8)

    CJ = C // 128   # routed token tiles
    QJ = TQ // 128  # shared token tiles
    FJ = F // 128   # hidden tiles

    d_xt32 = nc.dram_tensor("xt32", [128, 8, C], f32, kind="ExternalInput").ap()
    d_xtb = nc.dram_tensor("xtb", [128, 8, C], bf16, kind="ExternalInput").ap()
    d_xqb = nc.dram_tensor("xqb", [128, 8, TQ], bf16, kind="ExternalInput").ap()
    d_xres = nc.dram_tensor("x_res", [128, QJ, D], f32, kind="ExternalInput").ap()
    d_centT = nc.dram_tensor("centT", [128, 8, E], f32, kind="ExternalInput").ap()
    d_rbias = nc.dram_tensor("rbias", [128, E], f32, kind="ExternalInput").ap()
    d_sel = nc.dram_tensor("sel", [128, E], f32, kind="ExternalInput").ap()
    d_valid = nc.dram_tensor("valid", [128, CJ], f32, kind="ExternalInput").ap()
    d_ident = nc.dram_tensor("ident", [128, 128], f32, kind="ExternalInput").ap()
    d_wfc = nc.dram_tensor("wfcT", [128, 8, F], bf16, kind="ExternalInput").ap()
    d_wpj = nc.dram_tensor("wprojT", [128, FJ, D], bf16, kind="ExternalInput").ap()
    d_wfcs = nc.dram_tensor("wfcshT", [128, 8, FSH], bf16, kind="ExternalInput").ap()
    d_wpjs = nc.dram_tensor("wprojshT", [128, FJ, D], bf16, kind="ExternalInput").ap()
    d_outr = nc.dram_tensor("out_r", [128, CJ, D], f32, kind="ExternalOutput").ap()
    d_outs = nc.dram_tensor("out_sh", [128, QJ, D], f32, kind="ExternalOutput").ap()

    with tile.TileContext(nc) as tc, ExitStack() as ctx:
        const = ctx.enter_context(tc.tile_pool(name="const", bufs=1))
        xpool = ctx.enter_context(tc.tile_pool(name="xpool", bufs=1))
        ypool = ctx.enter_context(tc.tile_pool(name="ypool", bufs=1))
        rpool = ctx.enter_context(tc.tile_pool(name="rpool", bufs=2))
        wpool = ctx.enter_context(tc.tile_pool(name="wpool", bufs=2))
        hpool = ctx.enter_context(tc.tile_pool(name="hpool", bufs=2))
        php = ctx.enter_context(tc.tile_pool(name="php", bufs=2, space="PSUM"))
        pyp = ctx.enter_context(tc.tile_pool(name="pyp", bufs=2, space="PSUM"))

        loop_cm = (
            tc.For_i(0, R, 1, hint_engines=(ET.PE, ET.DVE, ET.SP, ET.Activation))
            if R > 1
            else nullcontext()
        )
        with loop_cm:
            # ---- resident loads ---------------------------------------------
            centT = const.tile([128, 8, E], f32, tag="cent")
            nc.sync.dma_start(centT[:], d_centT[:, :, :])
            rbias = const.tile([128, E], f32, tag="rbias")
            nc.sync.dma_start(rbias[:], d_rbias[:, :])
            sel = const.tile([128, E], f32, tag="sel")
            nc.sync.dma_start(sel[:], d_sel[:, :])
            valid = const.tile([128, CJ], f32, tag="valid")
            nc.sync.dma_start(valid[:], d_valid[:, :])
            ident = const.tile([128, 128], f32, tag="ident")
            nc.sync.dma_start(ident[:], d_ident[:, :])
            zbias = const.tile([128, 1], f32, tag="zbias")
            nc.vector.memset(zbias[:], 0.0)

            xgT = xpool.tile([128, 8, C], f32, tag="xgT")
            nc.sync.dma_start(xgT[:], d_xt32[:, :, :])
            xgb = xpool.tile([128, 8, C], bf16, tag="xgb")
            nc.sync.dma_start(xgb[:], d_xtb[:, :, :])
            xqb = xpool.tile([128, 8, TQ], bf16, tag="xqb")
            nc.sync.dma_start(xqb[:], d_xqb[:, :, :])
            xres = xpool.tile([128, QJ, D], f32, tag="xres")
            nc.sync.dma_start(xres[:], d_xres[:, :, :])

            y_r = ypool.tile([128, CJ, D], f32, tag="y_r")
            y_sh = ypool.tile([128, QJ, D], f32, tag="y_sh")
            ce = ypool.tile([128, CJ], f32, tag="ce")

            # ---- routing (fp32, device-side combine weights) ----------------
            raw_ps = php.tile([E, C], f32, tag="ph")
            for k in range(8):
                off = 0
                for w in _mchunks(C):
                    nc.tensor.matmul(
                        raw_ps[:, off:off + w],
                        lhsT=centT[:, k, :],
                        rhs=xgT[:, k, off:off + w],
                        start=(k == 0),
                        stop=(k == 7),
                    )
                    off += w
            raw_sb = rpool.tile([E, C], f32, tag="rawsb")
            nc.vector.tensor_copy(raw_sb[:], raw_ps[:])

            for j in range(CJ):
                t_ps = pyp.tile([128, E], f32, tag="py")
                nc.tensor.transpose(
                    t_ps[:], raw_sb[:, j * 128:(j + 1) * 128], ident[0:E, 0:E]
                )
                raw_t = rpool.tile([128, E], f32, tag="rawt")
                nc.vector.tensor_copy(raw_t[:], t_ps[:])
                braw = rpool.tile([128, E], f32, tag="braw")
                nc.vector.tensor_add(braw[:], raw_t[:], rbias[:])
                scores = rpool.tile([128, E], f32, tag="scores")
                nc.scalar.activation(scores[:], raw_t[:], AF.Sigmoid, bias=zbias[:])
                maxes = rpool.tile([128, 8], f32, tag="maxes")
                nc.vector.max(maxes[:], braw[:])
                mask1 = rpool.tile([128, E], f32, tag="mask1")
                nc.vector.tensor_tensor(
                    mask1[:], braw[:], maxes[:, 0:1].to_broadcast([128, E]), ALU.is_ge
                )
                mask2 = rpool.tile([128, E], f32, tag="mask2")
                nc.vector.tensor_tensor(
                    mask2[:], braw[:], maxes[:, 1:2].to_broadcast([128, E]), ALU.is_ge
                )
                nc.vector.tensor_sub(mask2[:], mask2[:], mask1[:])
                tmp = rpool.tile([128, E], f32, tag="tmp")
                s1 = rpool.tile([128, 1], f32, tag="s1")
                s2 = rpool.tile([128, 1], f32, tag="s2")
                nc.vector.tensor_mul(tmp[:], mask1[:], scores[:])
                nc.vector.tensor_reduce(s1[:], tmp[:], axis=AX.X, op=ALU.add)
                nc.vector.tensor_mul(tmp[:], mask2[:], scores[:])
                nc.vector.tensor_reduce(s2[:], tmp[:], axis=AX.X, op=ALU.add)
                i1 = rpool.tile([128, 1], f32, tag="i1")
                i2 = rpool.tile([128, 1], f32, tag="i2")
                nc.vector.tensor_mul(tmp[:], mask1[:], sel[:])
                nc.vector.tensor_reduce(i1[:], tmp[:], axis=AX.X, op=ALU.add)
                nc.vector.tensor_mul(tmp[:], mask2[:], sel[:])
                nc.vector.tensor_reduce(i2[:], tmp[:], axis=AX.X, op=ALU.add)
                w1 = rpool.tile([128, 1], f32, tag="w1")
                nc.vector.tensor_sub(s1[:], s1[:], s2[:])
                nc.scalar.activation(w1[:], s1[:], AF.Sigmoid, bias=zbias[:])
                w2 = rpool.tile([128, 1], f32, tag="w2")
                nc.vector.tensor_scalar(
                    w2[:], w1[:], -1.0, 1.0, op0=ALU.mult, op1=ALU.add
                )
                nc.vector.tensor_mul(i1[:], i1[:], w1[:])
                nc.vector.tensor_mul(i2[:], i2[:], w2[:])
                nc.vector.tensor_add(i1[:], i1[:], i2[:])
                nc.vector.tensor_mul(ce[:, j:j + 1], i1[:], valid[:, j:j + 1])

            # ---- MLP passes -------------------------------------------------
            def mlp(xsb, d_wfc_, d_wpj_, yacc, tcnt, fh):
                njt = tcnt // 128
                nch = fh // FCH
                mt = FCH // 128
                for ci in range(nch):
                    wfc_t = wpool.tile([128, 8, FCH], bf16, tag="wfc")
                    nc.sync.dma_start(
                        wfc_t[:], d_wfc_[:, :, ci * FCH:(ci + 1) * FCH]
                    )
                    wpj_t = wpool.tile([128, mt, D], bf16, tag="wpj")
                    nc.sync.dma_start(
                        wpj_t[:], d_wpj_[:, ci * mt:(ci + 1) * mt, :]
                    )
                    hs = []
                    for mi in range(mt):
                        ph = php.tile([128, tcnt], f32, tag="ph")
                        for k in range(8):
                            off = 0
                            for w in _mchunks(tcnt):
                                nc.tensor.matmul(
                                    ph[:, off:off + w],
                                    lhsT=wfc_t[:, k, mi * 128:(mi + 1) * 128],
                                    rhs=xsb[:, k, off:off + w],
                                    start=(k == 0),
                                    stop=(k == 7),
                                )
                                off += w
                        h = hpool.tile([128, tcnt], bf16, tag=f"h{mi}")
                        nc.scalar.activation(h[:], ph[:], AF.Gelu, bias=zbias[:])
                        hs.append(h)
                    for j in range(njt):
                        py = pyp.tile([128, D], f32, tag="py")
                        for mi in range(mt):
                            off = 0
                            for w in _mchunks(D):
                                nc.tensor.matmul(
                                    py[:, off:off + w],
                                    lhsT=hs[mi][:, j * 128:(j + 1) * 128],
                                    rhs=wpj_t[:, mi, off:off + w],
                                    start=(mi == 0),
                                    stop=(mi == mt - 1),
                                )
                                off += w
                        if ci == 0:
                            nc.vector.tensor_copy(yacc[:, j, :], py[:])
                        else:
                            nc.vector.tensor_add(yacc[:, j, :], yacc[:, j, :], py[:])

            mlp(xgb, d_wfc, d_wpj, y_r, C, F)
            mlp(xqb, d_wfcs, d_wpjs, y_sh, TQ, FSH)

            # ---- finalize + store -------------------------------------------
            for j in range(CJ):
                nc.vector.tensor_scalar_mul(y_r[:, j, :], y_r[:, j, :], ce[:, j:j + 1])
            nc.sync.dma_start(d_outr[:, :, :], y_r[:])
            for j in range(QJ):
                nc.vector.tensor_add(y_sh[:, j, :], y_sh[:, j, :], xres[:, j, :])
            nc.sync.dma_start(d_outs[:, :, :], y_sh[:])

    if not nc.is_finalized():
        nc.finalize()
    return nc


def _tile128(a):
    """[N, M] (N % 128 == 0) -> [128, N//128, M] partition-tiled layout."""
    n, m = a.shape
    return np.ascontiguousarray(a.reshape(n // 128, 128, m).transpose(1, 0, 2))


def kernel(x, centroids, routing_bias, Wfc_r, Wproj_r, Wfc_sh, Wproj_sh):
    from concourse import bass_utils

    x = np.asarray(x, np.float32)
    centroids = np.asarray(centroids, np.float32)
    routing_bias = np.asarray(routing_bias, np.float32)
    Wfc_r = np.asarray(Wfc_r, np.float32)
    Wproj_r = np.asarray(Wproj_r, np.float32)
    Wfc_sh = np.asarray(Wfc_sh, np.float32)
    Wproj_sh = np.asarray(Wproj_sh, np.float32)

    xf = np.ascontiguousarray(x.reshape(TT, D))

    # host routing — data placement only (device recomputes combine weights)
    raw = xf @ centroids.T
    balanced = raw + routing_bias[None, :]
    top2 = np.argsort(-balanced, axis=-1, kind="stable")[:, :2]
    idx_lists = []
    for e in range(E):
        hit = (top2 == e).any(axis=1)
        idx_lists.append(np.nonzero(hit)[0].astype(np.int64))
    nmax = max(len(ix) for ix in idx_lists)
    C = max(256, ((nmax + 127) // 128) * 128)

    if (C, 1) not in _prog_cache:
        _prog_cache[(C, 1)] = _build(C, 1)
    nc = _prog_cache[(C, 1)]

    xT = np.ascontiguousarray(xf.T)
    ident = np.eye(128, dtype=np.float32)
    rbias128 = np.tile(routing_bias[None, :], (128, 1)).astype(np.float32)
    FH = F // 2  # shared expert F-half

    in_maps = []
    for c in range(E):
        ix = idx_lists[c]
        n = len(ix)
        pad = np.zeros(C, np.int64)
        pad[:n] = ix
        xg = xf[pad]  # [C, D]
        xgT_ = np.ascontiguousarray(xg.T)  # [D, C]
        validm = np.zeros((128, C // 128), np.float32)
        for t in range(C):
            if t < n:
                validm[t % 128, t // 128] = 1.0
        sel = np.zeros((128, E), np.float32)
        sel[:, c] = 1.0
        q = c // 2
        half = c % 2
        wfcsh = np.concatenate(
            [Wfc_sh[nn, half * FH:(half + 1) * FH, :] for nn in range(NS)], axis=0
        )  # [FSH, D]
        wpjsh = np.concatenate(
            [Wproj_sh[nn, :, half * FH:(half + 1) * FH] for nn in range(NS)], axis=1
        )  # [D, FSH]
        xqT_ = np.ascontiguousarray(xT[:, q * TQ:(q + 1) * TQ])  # [D, TQ]
        in_maps.append({
            "xt32": _tile128(xgT_),
            "xtb": _tile128(xgT_).astype(BF16),
            "xqb": _tile128(xqT_).astype(BF16),
            "x_res": _tile128(np.ascontiguousarray(xf[q * TQ:(q + 1) * TQ] * 0.5)),
            "centT": _tile128(np.ascontiguousarray(centroids.T)),
            "rbias": rbias128,
            "sel": sel,
            "valid": validm,
            "ident": ident,
            "wfcT": _tile128(np.ascontiguousarray(Wfc_r[c].T)).astype(BF16),
            "wprojT": _tile128(np.ascontiguousarray(Wproj_r[c].T)).astype(BF16),
            "wfcshT": _tile128(np.ascontiguousarray(wfcsh.T)).astype(BF16),
            "wprojshT": _tile128(np.ascontiguousarray(wpjsh.T)).astype(BF16),
        })

    globals()["_last_in_maps"] = in_maps
    globals()["_last_C"] = C
    res = bass_utils.run_bass_kernel_spmd(nc, in_maps, core_ids=list(range(E)))
    globals()["_last_results"] = res

    out = np.zeros((TT, D), np.float32)
    for c in range(E):
        r = res.results[c]
        q = c // 2
        # [128, nj, D] tiled -> [nj*128, D] token-major
        out_sh = np.asarray(r["out_sh"]).transpose(1, 0, 2).reshape(TQ, D)
        out[q * TQ:(q + 1) * TQ] += out_sh
        out_r = np.asarray(r["out_r"]).transpose(1, 0, 2).reshape(C, D)
        ix = idx_lists[c]
        out[ix] += out_r[:len(ix)]
    return out.reshape(B, S, D)


# revision 3
# speedup vs baseline: 117.2768x; 1.0352x over previous
"""MoE kernel for Trainium2 — 8-core expert-parallel + shared-expert 2D shard.

Strategy:
  - Host computes routing (replica of reference math, fp32) ONLY to decide
    data placement: which tokens go to which expert-core (top-2 dispatch).
    The combine weights used in the output math are recomputed ON DEVICE
    from raw inputs (centroid matmul in fp32 + sigmoid/top2/softmax).
  - Core e (e=0..7): routed expert e's MLP over its gathered tokens
    (capacity-padded to C, invalid rows masked to 0 on device), plus a
    (token-quarter x F-half) shard of the 2 shared experts.
  - MLP matmuls in bf16 (full PE rate, half the HBM traffic); routing in
    exact fp32. PSUM accumulation is always fp32.
  - All DRAM operands use a [128, ntile, free] "partition-tiled" layout so
    each weight chunk is a single >=1MB dma_start.
  - The whole per-iteration body can be wrapped in a hardware For_i loop
    (R iterations); iterations are idempotent, so the R>1 program computes
    the same output while letting a single NEFF dispatch execute the kernel
    R times back-to-back (used to measure true per-execution HW time under
    a high fixed dispatch latency).
  - Host unshard: scatter-add routed partials (unique indices per core),
    sum shared partials; residual x folded in on-device via x_res halves.
"""

import sys

sys.path.insert(0, "/opt/trn_rl_repo")

import numpy as np
import ml_dtypes

BF16 = ml_dtypes.bfloat16

D = 1024
F = 4096          # routed expert hidden
FSH = 4096        # shared shard hidden: 2 experts x (4096/2) F-half
E = 8
NS = 2
B, S = 2, 1024
TT = B * S        # 2048 tokens
TQ = TT // 4      # 512-token quarter per shared shard
FCH = 1024        # hidden-chunk streamed per iteration

_prog_cache = {}


def _mchunks(n):
    """Split n into moving-dim chunks of 512 + remainder. Chunks must start
    at multiples of 512 so no matmul output crosses a PSUM bank boundary
    (fp32 bank = 512 floats)."""
    out = [512] * (n // 512)
    if n % 512:
        out.append(n % 512)
    return out


def _build(C, R=1):
    from contextlib import ExitStack, nullcontext
    from concourse import bacc, bass, tile, mybir

    f32 = mybir.dt.float32
    bf16 = mybir.dt.bfloat16
    AF = mybir.ActivationFunctionType
    ALU = mybir.AluOpType
    AX = mybir.AxisListType
    ET = mybir.EngineType

    nc = bacc.Bacc("TRN2", target_bir_lowering=False, debug=False, num_devices=8)

    CJ = C // 128   # routed token tiles
    QJ = TQ // 128  # shared token tiles
    FJ = F // 128   # hidden tiles

    d_xt32 = nc.dram_tensor("xt32", [128, 8, C], f32, kind="ExternalInput").ap()
    d_xtb = nc.dram_tensor("xtb", [128, 8, C], bf16, kind="ExternalInput").ap()
    d_xqb = nc.dram_tensor("xqb", [128, 8, TQ], bf16, kind="ExternalInput").ap()
    d_xres = nc.dram_tensor("x_res", [128, QJ, D], f32, kind="ExternalInput").ap()
    d_centT = nc.dram_tensor("centT", [128, 8, E], f32, kind="ExternalInput").ap()
    d_rbias = nc.dram_tensor("rbias", [128, E], f32, kind="ExternalInput").ap()
    d_sel = nc.dram_tensor("sel", [128, E], f32, kind="ExternalInput").ap()
    d_valid = nc.dram_tensor("valid", [128, CJ], f32, kind="ExternalInput").ap()
    d_ident = nc.dram_tensor("ident", [128, 128], f32, kind="ExternalInput").ap()
    d_wfc = nc.dram_tensor("wfcT", [128, 8, F], bf16, kind="ExternalInput").ap()
    d_wpj = nc.dram_tensor("wprojT", [128, FJ, D], bf16, kind="ExternalInput").ap()
    d_wfcs = nc.dram_tensor("wfcshT", [128, 8, FSH], bf16, kind="ExternalInput").ap()
    d_wpjs = nc.dram_tensor("wprojshT", [128, FJ, D], bf16, kind="ExternalInput").ap()
    d_outr = nc.dram_tensor("out_r", [128, CJ, D], f32, kind="ExternalOutput").ap()
    d_outs = nc.dram_tensor("out_sh", [128, QJ, D], f32, kind="ExternalOutput").ap()

    with tile.TileContext(nc) as tc, ExitStack() as ctx:
        const = ctx.enter_context(tc.tile_pool(name="const", bufs=1))
        xpool = ctx.enter_context(tc.tile_pool(name="xpool", bufs=1))
        ypool = ctx.enter_context(tc.tile_pool(name="ypool", bufs=1))
        rpool = ctx.enter_context(tc.tile_pool(name="rpool", bufs=2))
        wpool = ctx.enter_context(tc.tile_pool(name="wpool", bufs=2))
        hpool = ctx.enter_context(tc.tile_pool(name="hpool", bufs=2))
        php = ctx.enter_context(tc.tile_pool(name="php", bufs=2, space="PSUM"))
        pyp = ctx.enter_context(tc.tile_pool(name="pyp", bufs=2, space="PSUM"))

        loop_cm = (
            tc.For_i(0, R, 1, hint_engines=(ET.PE, ET.DVE, ET.SP, ET.Activation))
            if R > 1
            else nullcontext()
        )
        with loop_cm:
            # DMA issue order is SP-FIFO: the routed-MLP operands go first so
            # PE work starts as early as possible; routing inputs follow and
            # routing overlaps the shared MLP.
            zbias = const.tile([128, 1], f32, tag="zbias")
            nc.vector.memset(zbias[:], 0.0)
            xgb = xpool.tile([128, 8, C], bf16, tag="xgb")
            nc.sync.dma_start(xgb[:], d_xtb[:, :, :])

            y_r = ypool.tile([128, CJ, D], f32, tag="y_r")
            y_sh = ypool.tile([128, QJ, D], f32, tag="y_sh")
            ce = ypool.tile([128, CJ], f32, tag="ce")

            # ---- MLP (fc chunk ci overlaps proj of chunk ci-1) --------------
            def mlp(xsb, d_wfc_, d_wpj_, yacc, tcnt, fh, fin):
                njt = tcnt // 128
                nch = fh // FCH
                mt = FCH // 128

                def proj(ci, hs, wpj_t):
                    for j in range(njt):
                        py = pyp.tile([128, D], f32, tag="py")
                        for mi in range(mt):
                            off = 0
                            for w in _mchunks(D):
                                nc.tensor.matmul(
                                    py[:, off:off + w],
                                    lhsT=hs[mi][:, j * 128:(j + 1) * 128],
                                    rhs=wpj_t[:, mi, off:off + w],
                                    start=(mi == 0),
                                    stop=(mi == mt - 1),
                                )
                                off += w
                        if ci == 0:
                            nc.vector.tensor_copy(yacc[:, j, :], py[:])
                        else:
                            nc.vector.tensor_add(yacc[:, j, :], yacc[:, j, :], py[:])
                        if ci == nch - 1 and fin is not None:
                            fin(j)

                prev = None
                for ci in range(nch):
                    wfc_t = wpool.tile([128, 8, FCH], bf16, tag="wfc")
                    nc.sync.dma_start(
                        wfc_t[:], d_wfc_[:, :, ci * FCH:(ci + 1) * FCH]
                    )
                    wpj_t = wpool.tile([128, mt, D], bf16, tag="wpj")
                    nc.sync.dma_start(
                        wpj_t[:], d_wpj_[:, ci * mt:(ci + 1) * mt, :]
                    )
                    hs = []
                    for mi in range(mt):
                        ph = php.tile([128, tcnt], f32, tag="ph")
                        for k in range(8):
                            off = 0
                            for w in _mchunks(tcnt):
                                nc.tensor.matmul(
                                    ph[:, off:off + w],
                                    lhsT=wfc_t[:, k, mi * 128:(mi + 1) * 128],
                                    rhs=xsb[:, k, off:off + w],
                                    start=(k == 0),
                                    stop=(k == 7),
                                )
                                off += w
                        h = hpool.tile([128, tcnt], bf16, tag=f"h{mi}")
                        nc.scalar.activation(h[:], ph[:], AF.Gelu, bias=zbias[:])
                        hs.append(h)
                    if prev is not None:
                        proj(*prev)
                    prev = (ci, hs, wpj_t)
                proj(*prev)

            mlp(xgb, d_wfc, d_wpj, y_r, C, F, None)

            # ---- routing inputs + shared operands ---------------------------
            xqb = xpool.tile([128, 8, TQ], bf16, tag="xqb")
            nc.sync.dma_start(xqb[:], d_xqb[:, :, :])
            xres = xpool.tile([128, QJ, D], f32, tag="xres")
            nc.sync.dma_start(xres[:], d_xres[:, :, :])
            xgT = xpool.tile([128, 8, C], f32, tag="xgT")
            nc.sync.dma_start(xgT[:], d_xt32[:, :, :])
            centT = const.tile([128, 8, E], f32, tag="cent")
            nc.sync.dma_start(centT[:], d_centT[:, :, :])
            rbias = const.tile([128, E], f32, tag="rbias")
            nc.sync.dma_start(rbias[:], d_rbias[:, :])
            sel = const.tile([128, E], f32, tag="sel")
            nc.sync.dma_start(sel[:], d_sel[:, :])
            valid = const.tile([128, CJ], f32, tag="valid")
            nc.sync.dma_start(valid[:], d_valid[:, :])
            ident = const.tile([128, 128], f32, tag="ident")
            nc.sync.dma_start(ident[:], d_ident[:, :])

            # ---- routing (fp32, device-side combine weights); overlaps the
            # shared MLP on PE/DVE ---------------------------------------------
            raw_ps = pyp.tile([E, C], f32, tag="py")
            for k in range(8):
                off = 0
                for w in _mchunks(C):
                    nc.tensor.matmul(
                        raw_ps[:, off:off + w],
                        lhsT=centT[:, k, :],
                        rhs=xgT[:, k, off:off + w],
                        start=(k == 0),
                        stop=(k == 7),
                    )
                    off += w
            raw_sb = rpool.tile([E, C], f32, tag="rawsb")
            nc.vector.tensor_copy(raw_sb[:], raw_ps[:])

            for j in range(CJ):
                t_ps = pyp.tile([128, E], f32, tag="py")
                nc.tensor.transpose(
                    t_ps[:], raw_sb[:, j * 128:(j + 1) * 128], ident[0:E, 0:E]
                )
                raw_t = rpool.tile([128, E], f32, tag="rawt")
                nc.vector.tensor_copy(raw_t[:], t_ps[:])
                braw = rpool.tile([128, E], f32, tag="braw")
                nc.vector.tensor_add(braw[:], raw_t[:], rbias[:])
                scores = rpool.tile([128, E], f32, tag="scores")
                nc.scalar.activation(scores[:], raw_t[:], AF.Sigmoid, bias=zbias[:])
                maxes = rpool.tile([128, 8], f32, tag="maxes")
                nc.vector.max(maxes[:], braw[:])
                mask1 = rpool.tile([128, E], f32, tag="mask1")
                nc.vector.tensor_tensor(
                    mask1[:], braw[:], maxes[:, 0:1].to_broadcast([128, E]), ALU.is_ge
                )
                mask2 = rpool.tile([128, E], f32, tag="mask2")
                nc.vector.tensor_tensor(
                    mask2[:], braw[:], maxes[:, 1:2].to_broadcast([128, E]), ALU.is_ge
                )
                nc.vector.tensor_sub(mask2[:], mask2[:], mask1[:])
                tmp = rpool.tile([128, E], f32, tag="tmp")
                s1 = rpool.tile([128, 1], f32, tag="s1")
                s2 = rpool.tile([128, 1], f32, tag="s2")
                nc.vector.tensor_mul(tmp[:], mask1[:], scores[:])
                nc.vector.tensor_reduce(s1[:], tmp[:], axis=AX.X, op=ALU.add)
                nc.vector.tensor_mul(tmp[:], mask2[:], scores[:])
                nc.vector.tensor_reduce(s2[:], tmp[:], axis=AX.X, op=ALU.add)
                i1 = rpool.tile([128, 1], f32, tag="i1")
                i2 = rpool.tile([128, 1], f32, tag="i2")
                nc.vector.tensor_mul(tmp[:], mask1[:], sel[:])
                nc.vector.tensor_reduce(i1[:], tmp[:], axis=AX.X, op=ALU.add)
                nc.vector.tensor_mul(tmp[:], mask2[:], sel[:])
                nc.vector.tensor_reduce(i2[:], tmp[:], axis=AX.X, op=ALU.add)
                w1 = rpool.tile([128, 1], f32, tag="w1")
                nc.vector.tensor_sub(s1[:], s1[:], s2[:])
                nc.scalar.activation(w1[:], s1[:], AF.Sigmoid, bias=zbias[:])
                w2 = rpool.tile([128, 1], f32, tag="w2")
                nc.vector.tensor_scalar(
                    w2[:], w1[:], -1.0, 1.0, op0=ALU.mult, op1=ALU.add
                )
                nc.vector.tensor_mul(i1[:], i1[:], w1[:])
                nc.vector.tensor_mul(i2[:], i2[:], w2[:])
                nc.vector.tensor_add(i1[:], i1[:], i2[:])
                nc.vector.tensor_mul(ce[:, j:j + 1], i1[:], valid[:, j:j + 1])

            # routed finalize + store overlaps the shared MLP
            for j in range(CJ):
                nc.vector.tensor_scalar_mul(y_r[:, j, :], y_r[:, j, :], ce[:, j:j + 1])
                nc.sync.dma_start(d_outr[:, j, :], y_r[:, j, :])

            def fin_sh(j):
                nc.vector.tensor_add(y_sh[:, j, :], y_sh[:, j, :], xres[:, j, :])
                nc.sync.dma_start(d_outs[:, j, :], y_sh[:, j, :])

            mlp(xqb, d_wfcs, d_wpjs, y_sh, TQ, FSH, fin_sh)

    if not nc.is_finalized():
        nc.finalize()
    return nc


def _tile128(a):
    """[N, M] (N % 128 == 0) -> [128, N//128, M] partition-tiled layout."""
    n, m = a.shape
    return np.ascontiguousarray(a.reshape(n // 128, 128, m).transpose(1, 0, 2))


def kernel(x, centroids, routing_bias, Wfc_r, Wproj_r, Wfc_sh, Wproj_sh):
    from concourse import bass_utils

    x = np.asarray(x, np.float32)
    centroids = np.asarray(centroids, np.float32)
    routing_bias = np.asarray(routing_bias, np.float32)
    Wfc_r = np.asarray(Wfc_r, np.float32)
    Wproj_r = np.asarray(Wproj_r, np.float32)
    Wfc_sh = np.asarray(Wfc_sh, np.float32)
    Wproj_sh = np.asarray(Wproj_sh, np.float32)

    xf = np.ascontiguousarray(x.reshape(TT, D))

    # host routing — data placement only (device recomputes combine weights)
    raw = xf @ centroids.T
    balanced = raw + routing_bias[None, :]
    top2 = np.argsort(-balanced, axis=-1, kind="stable")[:, :2]
    idx_lists = []
    for e in range(E):
        hit = (top2 == e).any(axis=1)
        idx_lists.append(np.nonzero(hit)[0].astype(np.int64))
    nmax = max(len(ix) for ix in idx_lists)
    C = max(256, ((nmax + 127) // 128) * 128)

    if (C, 1) not in _prog_cache:
        _prog_cache[(C, 1)] = _build(C, 1)
    nc = _prog_cache[(C, 1)]

    xT = np.ascontiguousarray(xf.T)
    ident = np.eye(128, dtype=np.float32)
    rbias128 = np.tile(routing_bias[None, :], (128, 1)).astype(np.float32)
    FH = F // 2  # shared expert F-half

    in_maps = []
    for c in range(E):
        ix = idx_lists[c]
        n = len(ix)
        pad = np.zeros(C, np.int64)
        pad[:n] = ix
        xg = xf[pad]  # [C, D]
        xgT_ = np.ascontiguousarray(xg.T)  # [D, C]
        validm = np.zeros((128, C // 128), np.float32)
        for t in range(C):
            if t < n:
                validm[t % 128, t // 128] = 1.0
        sel = np.zeros((128, E), np.float32)
        sel[:, c] = 1.0
        q = c // 2
        half = c % 2
        wfcsh = np.concatenate(
            [Wfc_sh[nn, half * FH:(half + 1) * FH, :] for nn in range(NS)], axis=0
        )  # [FSH, D]
        wpjsh = np.concatenate(
            [Wproj_sh[nn, :, half * FH:(half + 1) * FH] for nn in range(NS)], axis=1
        )  # [D, FSH]
        xqT_ = np.ascontiguousarray(xT[:, q * TQ:(q + 1) * TQ])  # [D, TQ]
        in_maps.append({
            "xt32": _tile128(xgT_),
            "xtb": _tile128(xgT_).astype(BF16),
            "xqb": _tile128(xqT_).astype(BF16),
            "x_res": _tile128(np.ascontiguousarray(xf[q * TQ:(q + 1) * TQ] * 0.5)),
            "centT": _tile128(np.ascontiguousarray(centroids.T)),
            "rbias": rbias128,
            "sel": sel,
            "valid": validm,
            "ident": ident,
            "wfcT": _tile128(np.ascontiguousarray(Wfc_r[c].T)).astype(BF16),
            "wprojT": _tile128(np.ascontiguousarray(Wproj_r[c].T)).astype(BF16),
            "wfcshT": _tile128(np.ascontiguousarray(wfcsh.T)).astype(BF16),
            "wprojshT": _tile128(np.ascontiguousarray(wpjsh.T)).astype(BF16),
        })

    globals()["_last_in_maps"] = in_maps
    globals()["_last_C"] = C
    res = bass_utils.run_bass_kernel_spmd(nc, in_maps, core_ids=list(range(E)))
    globals()["_last_results"] = res

    out = np.zeros((TT, D), np.float32)
    for c in range(E):
        r = res.results[c]
        q = c // 2
        # [128, nj, D] tiled -> [nj*128, D] token-major
        out_sh = np.asarray(r["out_sh"]).transpose(1, 0, 2).reshape(TQ, D)
        out[q * TQ:(q + 1) * TQ] += out_sh
        out_r = np.asarray(r["out_r"]).transpose(1, 0, 2).reshape(C, D)
        ix = idx_lists[c]
        out[ix] += out_r[:len(ix)]
    return out.reshape(B, S, D)


# revision 14
# speedup vs baseline: 120.6367x; 1.0286x over previous
"""MoE kernel for Trainium2 — 8-core expert-parallel + shared-expert 2D shard.

Strategy:
  - Host computes routing (replica of reference math, fp32) ONLY to decide
    data placement: which tokens go to which expert-core (top-2 dispatch).
    The combine weights used in the output math are recomputed ON DEVICE
    from raw inputs (centroid matmul in fp32 + sigmoid/top2/softmax).
  - Core e (e=0..7): routed expert e's MLP over its gathered tokens
    (capacity-padded to C, invalid rows masked to 0 on device), plus a
    (token-quarter x F-half) shard of the 2 shared experts.
  - MLP matmuls in bf16 (full PE rate, half the HBM traffic); routing in
    exact fp32. PSUM accumulation is always fp32.
  - All DRAM operands use a [128, ntile, free] "partition-tiled" layout so
    each weight chunk is a single >=1MB dma_start.
  - The whole per-iteration body can be wrapped in a hardware For_i loop
    (R iterations); iterations are idempotent, so the R>1 program computes
    the same output while letting a single NEFF dispatch execute the kernel
    R times back-to-back (used to measure true per-execution HW time under
    a high fixed dispatch latency).
  - Host unshard: scatter-add routed partials (unique indices per core),
    sum shared partials; residual x folded in on-device via x_res halves.
"""

import sys

sys.path.insert(0, "/opt/trn_rl_repo")

import numpy as np
import ml_dtypes

BF16 = ml_dtypes.bfloat16

D = 1024
F = 4096          # routed expert hidden
FSH = 4096        # shared shard hidden: 2 experts x (4096/2) F-half
E = 8
NS = 2
B, S = 2, 1024
TT = B * S        # 2048 tokens
TQ = TT // 4      # 512-token quarter per shared shard
FCH = 1024        # hidden-chunk streamed per iteration

_prog_cache = {}


def _mchunks(n):
    """Split n into moving-dim chunks of 512 + remainder. Chunks must start
    at multiples of 512 so no matmul output crosses a PSUM bank boundary
    (fp32 bank = 512 floats)."""
    out = [512] * (n // 512)
    if n % 512:
        out.append(n % 512)
    return out


def _build(C, R=1):
    from contextlib import ExitStack, nullcontext
    from concourse import bacc, bass, tile, mybir

    f32 = mybir.dt.float32
    bf16 = mybir.dt.bfloat16
    AF = mybir.ActivationFunctionType
    ALU = mybir.AluOpType
    AX = mybir.AxisListType
    ET = mybir.EngineType

    nc = bacc.Bacc("TRN2", target_bir_lowering=False, debug=False, num_devices=8)

    CJ = C // 128   # routed token tiles
    QJ = TQ // 128  # shared token tiles
    FJ = F // 128   # hidden tiles

    d_xt32 = nc.dram_tensor("xt32", [128, 8, C], f32, kind="ExternalInput").ap()
    d_xtb = nc.dram_tensor("xtb", [128, 8, C], bf16, kind="ExternalInput").ap()
    d_xqb = nc.dram_tensor("xqb", [128, 8, TQ], bf16, kind="ExternalInput").ap()
    d_xres = nc.dram_tensor("x_res", [128, QJ, D], f32, kind="ExternalInput").ap()
    d_centT = nc.dram_tensor("centT", [128, 8, E], f32, kind="ExternalInput").ap()
    d_rbias = nc.dram_tensor("rbias", [128, E], f32, kind="ExternalInput").ap()
    d_sel = nc.dram_tensor("sel", [128, E], f32, kind="ExternalInput").ap()
    d_valid = nc.dram_tensor("valid", [128, CJ], f32, kind="ExternalInput").ap()
    d_ident = nc.dram_tensor("ident", [128, 128], f32, kind="ExternalInput").ap()
    d_wfc = nc.dram_tensor("wfcT", [128, 8, F], bf16, kind="ExternalInput").ap()
    d_wpj = nc.dram_tensor("wprojT", [128, FJ, D], bf16, kind="ExternalInput").ap()
    d_wfcs = nc.dram_tensor("wfcshT", [128, 8, FSH], bf16, kind="ExternalInput").ap()
    d_wpjs = nc.dram_tensor("wprojshT", [128, FJ, D], bf16, kind="ExternalInput").ap()
    d_outr = nc.dram_tensor("out_r", [128, CJ, D], f32, kind="ExternalOutput").ap()
    d_outs = nc.dram_tensor("out_sh", [128, QJ, D], f32, kind="ExternalOutput").ap()

    with tile.TileContext(nc) as tc, ExitStack() as ctx:
        const = ctx.enter_context(tc.tile_pool(name="const", bufs=1))
        xpool = ctx.enter_context(tc.tile_pool(name="xpool", bufs=1))
        ypool = ctx.enter_context(tc.tile_pool(name="ypool", bufs=1))
        rpool = ctx.enter_context(tc.tile_pool(name="rpool", bufs=2))
        wpool = ctx.enter_context(tc.tile_pool(name="wpool", bufs=2))
        hpool = ctx.enter_context(tc.tile_pool(name="hpool", bufs=2))
        php = ctx.enter_context(tc.tile_pool(name="php", bufs=2, space="PSUM"))
        pyp = ctx.enter_context(tc.tile_pool(name="pyp", bufs=4, space="PSUM"))

        loop_cm = (
            tc.For_i(0, R, 1, hint_engines=(ET.PE, ET.DVE, ET.SP, ET.Activation))
            if R > 1
            else nullcontext()
        )
        with loop_cm:
            # DMA issue order is SP-FIFO: the routed-MLP operands go first so
            # PE work starts as early as possible; routing inputs follow and
            # routing overlaps the shared MLP.
            zbias = const.tile([128, 1], f32, tag="zbias")
            nc.vector.memset(zbias[:], 0.0)
            xgb = xpool.tile([128, 8, C], bf16, tag="xgb")

            y_r = ypool.tile([128, CJ, D], f32, tag="y_r")
            y_sh = ypool.tile([128, QJ, D], f32, tag="y_sh")
            ce = ypool.tile([128, CJ], f32, tag="ce")

            # ---- MLP (fc chunk ci overlaps proj of chunk ci-1) --------------
            def mlp(xsb, d_wfc_, d_wpj_, yacc, tcnt, fh, fin, x_dma=None,
                    init=None):
                njt = tcnt // 128
                nch = fh // FCH
                mt = FCH // 128

                def proj(ci, hs, wpj_t):
                    for j in range(njt):
                        pys = [
                            pyp.tile([128, 512], f32, tag="py", name=f"py{hi}")
                            for hi in range(D // 512)
                        ]
                        for mi in range(mt):
                            for hi, py in enumerate(pys):
                                nc.tensor.matmul(
                                    py[:],
                                    lhsT=hs[mi][:, j * 128:(j + 1) * 128],
                                    rhs=wpj_t[:, mi, hi * 512:(hi + 1) * 512],
                                    start=(mi == 0),
                                    stop=(mi == mt - 1),
                                )
                        for hi, py in enumerate(pys):
                            dst = yacc[:, j, hi * 512:(hi + 1) * 512]
                            if ci == 0:
                                if init is not None:
                                    nc.vector.tensor_add(
                                        dst, py[:], init[:, j, hi * 512:(hi + 1) * 512]
                                    )
                                else:
                                    nc.vector.tensor_copy(dst, py[:])
                            else:
                                nc.vector.tensor_add(dst, dst, py[:])
                        if ci == nch - 1 and fin is not None:
                            fin(j)

                prev = None
                for ci in range(nch):
                    wfc_t = wpool.tile([128, 8, FCH], bf16, tag="wfc")
                    if ci == 0:
                        # interleave the per-k x and weight loads so the first
                        # matmul's operands land first on the DMA rings
                        for di in range(8):
                            if x_dma is not None:
                                nc.sync.dma_start(xsb[:, di, :], x_dma[:, di, :])
                            nc.sync.dma_start(
                                wfc_t[:, di, :],
                                d_wfc_[:, di, ci * FCH:(ci + 1) * FCH],
                            )
                    else:
                        nc.sync.dma_start(
                            wfc_t[:], d_wfc_[:, :, ci * FCH:(ci + 1) * FCH]
                        )
                    wpj_t = wpool.tile([128, mt, D], bf16, tag="wpj")
                    nc.sync.dma_start(
                        wpj_t[:], d_wpj_[:, ci * mt:(ci + 1) * mt, :]
                    )
                    hs = []
                    for mi in range(mt):
                        ph = php.tile([128, tcnt], f32, tag="ph")
                        for k in range(8):
                            off = 0
                            for w in _mchunks(tcnt):
                                nc.tensor.matmul(
                                    ph[:, off:off + w],
                                    lhsT=wfc_t[:, k, mi * 128:(mi + 1) * 128],
                                    rhs=xsb[:, k, off:off + w],
                                    start=(k == 0),
                                    stop=(k == 7),
                                )
                                off += w
                        h = hpool.tile([128, tcnt], bf16, tag=f"h{mi}")
                        nc.scalar.activation(h[:], ph[:], AF.Gelu, bias=zbias[:])
                        hs.append(h)
                    if prev is not None:
                        proj(*prev)
                    prev = (ci, hs, wpj_t)
                proj(*prev)

            mlp(xgb, d_wfc, d_wpj, y_r, C, F, None, x_dma=d_xtb)

            # ---- routing inputs + shared operands ---------------------------
            xqb = xpool.tile([128, 8, TQ], bf16, tag="xqb")
            nc.sync.dma_start(xqb[:], d_xqb[:, :, :])
            xres = xpool.tile([128, QJ, D], f32, tag="xres")
            nc.sync.dma_start(xres[:], d_xres[:, :, :])
            xgT = xpool.tile([128, 8, C], f32, tag="xgT")
            nc.sync.dma_start(xgT[:], d_xt32[:, :, :])
            centT = const.tile([128, 8, E], f32, tag="cent")
            nc.sync.dma_start(centT[:], d_centT[:, :, :])
            rbias = const.tile([128, E], f32, tag="rbias")
            nc.sync.dma_start(rbias[:], d_rbias[:, :])
            sel = const.tile([128, E], f32, tag="sel")
            nc.sync.dma_start(sel[:], d_sel[:, :])
            valid = const.tile([128, CJ], f32, tag="valid")
            nc.sync.dma_start(valid[:], d_valid[:, :])
            ident = const.tile([128, 128], f32, tag="ident")
            nc.sync.dma_start(ident[:], d_ident[:, :])

            # ---- routing (fp32, device-side combine weights); overlaps the
            # shared MLP on PE/DVE ---------------------------------------------
            raw_ps = php.tile([E, C], f32, tag="ph")
            for k in range(8):
                off = 0
                for w in _mchunks(C):
                    nc.tensor.matmul(
                        raw_ps[:, off:off + w],
                        lhsT=centT[:, k, :],
                        rhs=xgT[:, k, off:off + w],
                        start=(k == 0),
                        stop=(k == 7),
                    )
                    off += w
            raw_sb = rpool.tile([E, C], f32, tag="rawsb")
            nc.vector.tensor_copy(raw_sb[:], raw_ps[:])

            for j in range(CJ):
                t_ps = pyp.tile([128, E], f32, tag="py")
                nc.tensor.transpose(
                    t_ps[:], raw_sb[:, j * 128:(j + 1) * 128], ident[0:E, 0:E]
                )
                raw_t = rpool.tile([128, E], f32, tag="rawt")
                nc.vector.tensor_copy(raw_t[:], t_ps[:])
                braw = rpool.tile([128, E], f32, tag="braw")
                nc.vector.tensor_add(braw[:], raw_t[:], rbias[:])
                scores = rpool.tile([128, E], f32, tag="scores")
                nc.scalar.activation(scores[:], raw_t[:], AF.Sigmoid, bias=zbias[:])
                maxes = rpool.tile([128, 8], f32, tag="maxes")
                nc.vector.max(maxes[:], braw[:])
                mask1 = rpool.tile([128, E], f32, tag="mask1")
                nc.vector.tensor_tensor(
                    mask1[:], braw[:], maxes[:, 0:1].to_broadcast([128, E]), ALU.is_ge
                )
                mask2 = rpool.tile([128, E], f32, tag="mask2")
                nc.vector.tensor_tensor(
                    mask2[:], braw[:], maxes[:, 1:2].to_broadcast([128, E]), ALU.is_ge
                )
                nc.vector.tensor_sub(mask2[:], mask2[:], mask1[:])
                tmp = rpool.tile([128, E], f32, tag="tmp")
                s1 = rpool.tile([128, 1], f32, tag="s1")
                s2 = rpool.tile([128, 1], f32, tag="s2")
                nc.vector.tensor_mul(tmp[:], mask1[:], scores[:])
                nc.vector.tensor_reduce(s1[:], tmp[:], axis=AX.X, op=ALU.add)
                nc.vector.tensor_mul(tmp[:], mask2[:], scores[:])
                nc.vector.tensor_reduce(s2[:], tmp[:], axis=AX.X, op=ALU.add)
                i1 = rpool.tile([128, 1], f32, tag="i1")
                i2 = rpool.tile([128, 1], f32, tag="i2")
                nc.vector.tensor_mul(tmp[:], mask1[:], sel[:])
                nc.vector.tensor_reduce(i1[:], tmp[:], axis=AX.X, op=ALU.add)
                nc.vector.tensor_mul(tmp[:], mask2[:], sel[:])
                nc.vector.tensor_reduce(i2[:], tmp[:], axis=AX.X, op=ALU.add)
                w1 = rpool.tile([128, 1], f32, tag="w1")
                nc.vector.tensor_sub(s1[:], s1[:], s2[:])
                nc.scalar.activation(w1[:], s1[:], AF.Sigmoid, bias=zbias[:])
                w2 = rpool.tile([128, 1], f32, tag="w2")
                nc.vector.tensor_scalar(
                    w2[:], w1[:], -1.0, 1.0, op0=ALU.mult, op1=ALU.add
                )
                nc.vector.tensor_mul(i1[:], i1[:], w1[:])
                nc.vector.tensor_mul(i2[:], i2[:], w2[:])
                nc.vector.tensor_add(i1[:], i1[:], i2[:])
                nc.vector.tensor_mul(ce[:, j:j + 1], i1[:], valid[:, j:j + 1])

            # routed finalize + store overlaps the shared MLP
            for j in range(CJ):
                nc.vector.tensor_scalar_mul(y_r[:, j, :], y_r[:, j, :], ce[:, j:j + 1])
                nc.sync.dma_start(d_outr[:, j, :], y_r[:, j, :])

            def fin_sh(j):
                nc.sync.dma_start(d_outs[:, j, :], y_sh[:, j, :])

            mlp(xqb, d_wfcs, d_wpjs, y_sh, TQ, FSH, fin_sh, init=xres)

    if not nc.is_finalized():
        nc.finalize()
    return nc


def _tile128(a):
    """[N, M] (N % 128 == 0) -> [128, N//128, M] partition-tiled layout."""
    n, m = a.shape
    return np.ascontiguousarray(a.reshape(n // 128, 128, m).transpose(1, 0, 2))


def kernel(x, centroids, routing_bias, Wfc_r, Wproj_r, Wfc_sh, Wproj_sh):
    from concourse import bass_utils

    x = np.asarray(x, np.float32)
    centroids = np.asarray(centroids, np.float32)
    routing_bias = np.asarray(routing_bias, np.float32)
    Wfc_r = np.asarray(Wfc_r, np.float32)
    Wproj_r = np.asarray(Wproj_r, np.float32)
    Wfc_sh = np.asarray(Wfc_sh, np.float32)
    Wproj_sh = np.asarray(Wproj_sh, np.float32)

    xf = np.ascontiguousarray(x.reshape(TT, D))

    # host routing — data placement only (device recomputes combine weights)
    raw = xf @ centroids.T
    balanced = raw + routing_bias[None, :]
    top2 = np.argsort(-balanced, axis=-1, kind="stable")[:, :2]
    idx_lists = []
    for e in range(E):
        hit = (top2 == e).any(axis=1)
        idx_lists.append(np.nonzero(hit)[0].astype(np.int64))
    nmax = max(len(ix) for ix in idx_lists)
    C = max(256, ((nmax + 127) // 128) * 128)

    if (C, 1) not in _prog_cache:
        _prog_cache[(C, 1)] = _build(C, 1)
    nc = _prog_cache[(C, 1)]

    xT = np.ascontiguousarray(xf.T)
    ident = np.eye(128, dtype=np.float32)
    rbias128 = np.tile(routing_bias[None, :], (128, 1)).astype(np.float32)
    FH = F // 2  # shared expert F-half

    in_maps = []
    for c in range(E):
        ix = idx_lists[c]
        n = len(ix)
        pad = np.zeros(C, np.int64)
        pad[:n] = ix
        xg = xf[pad]  # [C, D]
        xgT_ = np.ascontiguousarray(xg.T)  # [D, C]
        validm = np.zeros((128, C // 128), np.float32)
        for t in range(C):
            if t < n:
                validm[t % 128, t // 128] = 1.0
        sel = np.zeros((128, E), np.float32)
        sel[:, c] = 1.0
        q = c // 2
        half = c % 2
        wfcsh = np.concatenate(
            [Wfc_sh[nn, half * FH:(half + 1) * FH, :] for nn in range(NS)], axis=0
        )  # [FSH, D]
        wpjsh = np.concatenate(
            [Wproj_sh[nn, :, half * FH:(half + 1) * FH] for nn in range(NS)], axis=1
        )  # [D, FSH]
        xqT_ = np.ascontiguousarray(xT[:, q * TQ:(q + 1) * TQ])  # [D, TQ]
        in_maps.append({
            "xt32": _tile128(xgT_),
            "xtb": _tile128(xgT_).astype(BF16),
            "xqb": _tile128(xqT_).astype(BF16),
            "x_res": _tile128(np.ascontiguousarray(xf[q * TQ:(q + 1) * TQ] * 0.5)),
            "centT": _tile128(np.ascontiguousarray(centroids.T)),
            "rbias": rbias128,
            "sel": sel,
            "valid": validm,
            "ident": ident,
            "wfcT": _tile128(np.ascontiguousarray(Wfc_r[c].T)).astype(BF16),
            "wprojT": _tile128(np.ascontiguousarray(Wproj_r[c].T)).astype(BF16),
            "wfcshT": _tile128(np.ascontiguousarray(wfcsh.T)).astype(BF16),
            "wprojshT": _tile128(np.ascontiguousarray(wpjsh.T)).astype(BF16),
        })

    globals()["_last_in_maps"] = in_maps
    globals()["_last_C"] = C
    res = bass_utils.run_bass_kernel_spmd(nc, in_maps, core_ids=list(range(E)))
    globals()["_last_results"] = res

    out = np.zeros((TT, D), np.float32)
    for c in range(E):
        r = res.results[c]
        q = c // 2
        # [128, nj, D] tiled -> [nj*128, D] token-major
        out_sh = np.asarray(r["out_sh"]).transpose(1, 0, 2).reshape(TQ, D)
        out[q * TQ:(q + 1) * TQ] += out_sh
        out_r = np.asarray(r["out_r"]).transpose(1, 0, 2).reshape(C, D)
        ix = idx_lists[c]
        out[ix] += out_r[:len(ix)]
    return out.reshape(B, S, D)


# revision 16
# speedup vs baseline: 121.1168x; 1.0040x over previous
"""MoE kernel for Trainium2 — 8-core expert-parallel + shared-expert 2D shard.

Strategy:
  - Host computes routing (replica of reference math, fp32) ONLY to decide
    data placement: which tokens go to which expert-core (top-2 dispatch).
    The combine weights used in the output math are recomputed ON DEVICE
    from raw inputs (centroid matmul in fp32 + sigmoid/top2/softmax).
  - Core e (e=0..7): routed expert e's MLP over its gathered tokens
    (capacity-padded to C, invalid rows masked to 0 on device), plus a
    (token-quarter x F-half) shard of the 2 shared experts.
  - MLP matmuls in bf16 (full PE rate, half the HBM traffic); routing in
    exact fp32. PSUM accumulation is always fp32.
  - All DRAM operands use a [128, ntile, free] "partition-tiled" layout so
    each weight chunk is a single >=1MB dma_start.
  - The whole per-iteration body can be wrapped in a hardware For_i loop
    (R iterations); iterations are idempotent, so the R>1 program computes
    the same output while letting a single NEFF dispatch execute the kernel
    R times back-to-back (used to measure true per-execution HW time under
    a high fixed dispatch latency).
  - Host unshard: scatter-add routed partials (unique indices per core),
    sum shared partials; residual x folded in on-device via x_res halves.
"""

import sys

sys.path.insert(0, "/opt/trn_rl_repo")

import numpy as np
import ml_dtypes

BF16 = ml_dtypes.bfloat16

D = 1024
F = 4096          # routed expert hidden
FSH = 4096        # shared shard hidden: 2 experts x (4096/2) F-half
E = 8
NS = 2
B, S = 2, 1024
TT = B * S        # 2048 tokens
TQ = TT // 4      # 512-token quarter per shared shard
FCH = 1024        # hidden-chunk streamed per iteration

_prog_cache = {}


def _mchunks(n):
    """Split n into moving-dim chunks of 512 + remainder. Chunks must start
    at multiples of 512 so no matmul output crosses a PSUM bank boundary
    (fp32 bank = 512 floats)."""
    out = [512] * (n // 512)
    if n % 512:
        out.append(n % 512)
    return out


def _build(C, R=1):
    from contextlib import ExitStack, nullcontext
    from concourse import bacc, bass, tile, mybir

    f32 = mybir.dt.float32
    bf16 = mybir.dt.bfloat16
    AF = mybir.ActivationFunctionType
    ALU = mybir.AluOpType
    AX = mybir.AxisListType
    ET = mybir.EngineType

    nc = bacc.Bacc("TRN2", target_bir_lowering=False, debug=False, num_devices=8)

    CJ = C // 128   # routed token tiles
    QJ = TQ // 128  # shared token tiles
    FJ = F // 128   # hidden tiles

    d_xt32 = nc.dram_tensor("xt32", [128, 8, C], f32, kind="ExternalInput").ap()
    d_xtb = nc.dram_tensor("xtb", [128, 8, C], bf16, kind="ExternalInput").ap()
    d_xqb = nc.dram_tensor("xqb", [128, 8, TQ], bf16, kind="ExternalInput").ap()
    d_xres = nc.dram_tensor("x_res", [128, QJ, D], f32, kind="ExternalInput").ap()
    d_centT = nc.dram_tensor("centT", [128, 8, E], f32, kind="ExternalInput").ap()
    d_rbias = nc.dram_tensor("rbias", [128, E], f32, kind="ExternalInput").ap()
    d_sel = nc.dram_tensor("sel", [128, E], f32, kind="ExternalInput").ap()
    d_valid = nc.dram_tensor("valid", [128, CJ], f32, kind="ExternalInput").ap()
    d_ident = nc.dram_tensor("ident", [128, 128], f32, kind="ExternalInput").ap()
    d_wfc = nc.dram_tensor("wfcT", [128, 8, F], bf16, kind="ExternalInput").ap()
    d_wpj = nc.dram_tensor("wprojT", [128, FJ, D], bf16, kind="ExternalInput").ap()
    d_wfcs = nc.dram_tensor("wfcshT", [128, 8, FSH], bf16, kind="ExternalInput").ap()
    d_wpjs = nc.dram_tensor("wprojshT", [128, FJ, D], bf16, kind="ExternalInput").ap()
    d_outr = nc.dram_tensor("out_r", [128, CJ, D], f32, kind="ExternalOutput").ap()
    d_outs = nc.dram_tensor("out_sh", [128, QJ, D], f32, kind="ExternalOutput").ap()

    with tile.TileContext(nc) as tc, ExitStack() as ctx:
        const = ctx.enter_context(tc.tile_pool(name="const", bufs=1))
        xpool = ctx.enter_context(tc.tile_pool(name="xpool", bufs=1))
        ypool = ctx.enter_context(tc.tile_pool(name="ypool", bufs=1))
        rpool = ctx.enter_context(tc.tile_pool(name="rpool", bufs=2))
        wpool = ctx.enter_context(tc.tile_pool(name="wpool", bufs=2))
        hpool = ctx.enter_context(tc.tile_pool(name="hpool", bufs=2))
        php = ctx.enter_context(tc.tile_pool(name="php", bufs=2, space="PSUM"))
        pyp = ctx.enter_context(tc.tile_pool(name="pyp", bufs=4, space="PSUM"))

        loop_cm = (
            tc.For_i(0, R, 1, hint_engines=(ET.PE, ET.DVE, ET.SP, ET.Activation))
            if R > 1
            else nullcontext()
        )
        with loop_cm:
            # DMA issue order is SP-FIFO: the routed-MLP operands go first so
            # PE work starts as early as possible; routing inputs follow and
            # routing overlaps the shared MLP.
            zbias = const.tile([128, 1], f32, tag="zbias")
            nc.vector.memset(zbias[:], 0.0)
            xgb = xpool.tile([128, 8, C], bf16, tag="xgb")

            y_r = ypool.tile([128, CJ, D], f32, tag="y_r")
            y_sh = ypool.tile([128, QJ, D], f32, tag="y_sh")
            ce = ypool.tile([128, CJ], f32, tag="ce")

            # ---- MLP (fc chunk ci overlaps proj of chunk ci-1) --------------
            def mlp(xsb, d_wfc_, d_wpj_, yacc, tcnt, fh, fin, x_dma=None,
                    init=None):
                njt = tcnt // 128
                nch = fh // FCH
                mt = FCH // 128

                def proj(ci, hs, wpj_t):
                    for j in range(njt):
                        pys = [
                            pyp.tile([128, 512], f32, tag="py", name=f"py{hi}")
                            for hi in range(D // 512)
                        ]
                        for mi in range(mt):
                            for hi, py in enumerate(pys):
                                nc.tensor.matmul(
                                    py[:],
                                    lhsT=hs[mi][:, j * 128:(j + 1) * 128],
                                    rhs=wpj_t[:, mi, hi * 512:(hi + 1) * 512],
                                    start=(mi == 0),
                                    stop=(mi == mt - 1),
                                )
                        for hi, py in enumerate(pys):
                            dst = yacc[:, j, hi * 512:(hi + 1) * 512]
                            if ci == 0:
                                if init is not None:
                                    nc.vector.tensor_add(
                                        dst, py[:], init[:, j, hi * 512:(hi + 1) * 512]
                                    )
                                else:
                                    nc.vector.tensor_copy(dst, py[:])
                            else:
                                nc.vector.tensor_add(dst, dst, py[:])
                        if ci == nch - 1 and fin is not None:
                            fin(j)

                prev = None
                for ci in range(nch):
                    wfc_t = wpool.tile([128, 8, FCH], bf16, tag="wfc")
                    if ci == 0:
                        # interleave the per-k x and weight loads so the first
                        # matmul's operands land first on the DMA rings
                        for di in range(8):
                            if x_dma is not None:
                                nc.sync.dma_start(xsb[:, di, :], x_dma[:, di, :])
                            nc.sync.dma_start(
                                wfc_t[:, di, :],
                                d_wfc_[:, di, ci * FCH:(ci + 1) * FCH],
                            )
                    else:
                        nc.sync.dma_start(
                            wfc_t[:], d_wfc_[:, :, ci * FCH:(ci + 1) * FCH]
                        )
                    wpj_t = wpool.tile([128, mt, D], bf16, tag="wpj")
                    nc.sync.dma_start(
                        wpj_t[:], d_wpj_[:, ci * mt:(ci + 1) * mt, :]
                    )
                    hs = []
                    for mi in range(mt):
                        ph = php.tile([128, tcnt], f32, tag="ph")
                        for k in range(8):
                            off = 0
                            for w in _mchunks(tcnt):
                                nc.tensor.matmul(
                                    ph[:, off:off + w],
                                    lhsT=wfc_t[:, k, mi * 128:(mi + 1) * 128],
                                    rhs=xsb[:, k, off:off + w],
                                    start=(k == 0),
                                    stop=(k == 7),
                                )
                                off += w
                        h = hpool.tile([128, tcnt], bf16, tag=f"h{mi}")
                        nc.scalar.activation(h[:], ph[:], AF.Gelu, bias=zbias[:])
                        hs.append(h)
                    if prev is not None:
                        proj(*prev)
                    prev = (ci, hs, wpj_t)
                proj(*prev)

            mlp(xgb, d_wfc, d_wpj, y_r, C, F, None, x_dma=d_xtb)

            # ---- routing inputs + shared operands ---------------------------
            xqb = xpool.tile([128, 8, TQ], bf16, tag="xqb")
            nc.sync.dma_start(xqb[:], d_xqb[:, :, :])
            xres = xpool.tile([128, QJ, D], f32, tag="xres")
            nc.sync.dma_start(xres[:], d_xres[:, :, :])
            xgT = xpool.tile([128, 8, C], f32, tag="xgT")
            nc.sync.dma_start(xgT[:], d_xt32[:, :, :])
            centT = const.tile([128, 8, E], f32, tag="cent")
            nc.sync.dma_start(centT[:], d_centT[:, :, :])
            rbias = const.tile([128, E], f32, tag="rbias")
            nc.sync.dma_start(rbias[:], d_rbias[:, :])
            sel = const.tile([128, E], f32, tag="sel")
            nc.sync.dma_start(sel[:], d_sel[:, :])
            valid = const.tile([128, CJ], f32, tag="valid")
            nc.sync.dma_start(valid[:], d_valid[:, :])
            ident = const.tile([128, 128], f32, tag="ident")
            nc.sync.dma_start(ident[:], d_ident[:, :])

            # ---- routing (fp32, device-side combine weights); overlaps the
            # shared MLP on PE/DVE ---------------------------------------------
            raw_ps = php.tile([E, C], f32, tag="ph")
            for k in range(8):
                off = 0
                for w in _mchunks(C):
                    nc.tensor.matmul(
                        raw_ps[:, off:off + w],
                        lhsT=centT[:, k, :],
                        rhs=xgT[:, k, off:off + w],
                        start=(k == 0),
                        stop=(k == 7),
                    )
                    off += w
            raw_sb = rpool.tile([E, C], f32, tag="rawsb")
            nc.vector.tensor_copy(raw_sb[:], raw_ps[:])

            for j in range(CJ):
                t_ps = pyp.tile([128, E], f32, tag="py")
                nc.tensor.transpose(
                    t_ps[:], raw_sb[:, j * 128:(j + 1) * 128], ident[0:E, 0:E]
                )
                raw_t = rpool.tile([128, E], f32, tag="rawt")
                nc.vector.tensor_copy(raw_t[:], t_ps[:])
                braw = rpool.tile([128, E], f32, tag="braw")
                nc.vector.tensor_add(braw[:], raw_t[:], rbias[:])
                # sigmoid via the tanh entry of the gelu table: keeping every
                # activation on one act-func set hoists the table load out of
                # the loop (a mid-iteration table switch stalls ACT ~1.3us)
                scores = rpool.tile([128, E], f32, tag="scores")
                nc.scalar.activation(
                    scores[:], raw_t[:], AF.Tanh, bias=zbias[:], scale=0.5
                )
                maxes = rpool.tile([128, 8], f32, tag="maxes")
                nc.vector.max(maxes[:], braw[:])
                mask1 = rpool.tile([128, E], f32, tag="mask1")
                nc.vector.tensor_tensor(
                    mask1[:], braw[:], maxes[:, 0:1].to_broadcast([128, E]), ALU.is_ge
                )
                mask2 = rpool.tile([128, E], f32, tag="mask2")
                nc.vector.tensor_tensor(
                    mask2[:], braw[:], maxes[:, 1:2].to_broadcast([128, E]), ALU.is_ge
                )
                nc.vector.tensor_sub(mask2[:], mask2[:], mask1[:])
                tmp = rpool.tile([128, E], f32, tag="tmp")
                s1 = rpool.tile([128, 1], f32, tag="s1")
                s2 = rpool.tile([128, 1], f32, tag="s2")
                nc.vector.tensor_mul(tmp[:], mask1[:], scores[:])
                nc.vector.tensor_reduce(s1[:], tmp[:], axis=AX.X, op=ALU.add)
                nc.vector.tensor_mul(tmp[:], mask2[:], scores[:])
                nc.vector.tensor_reduce(s2[:], tmp[:], axis=AX.X, op=ALU.add)
                i1 = rpool.tile([128, 1], f32, tag="i1")
                i2 = rpool.tile([128, 1], f32, tag="i2")
                nc.vector.tensor_mul(tmp[:], mask1[:], sel[:])
                nc.vector.tensor_reduce(i1[:], tmp[:], axis=AX.X, op=ALU.add)
                nc.vector.tensor_mul(tmp[:], mask2[:], sel[:])
                nc.vector.tensor_reduce(i2[:], tmp[:], axis=AX.X, op=ALU.add)
                # here scores = tanh(raw/2) = 2*sigmoid(raw)-1 and each mask
                # selects exactly one expert, so sig1-sig2 = (s1-s2)/2 and
                # w1 = sigmoid(sig1-sig2) = 0.5 + 0.5*tanh((s1-s2)/4)
                w1 = rpool.tile([128, 1], f32, tag="w1")
                nc.vector.tensor_sub(s1[:], s1[:], s2[:])
                nc.scalar.activation(
                    w1[:], s1[:], AF.Tanh, bias=zbias[:], scale=0.25
                )
                nc.vector.tensor_scalar(
                    w1[:], w1[:], 0.5, 0.5, op0=ALU.mult, op1=ALU.add
                )
                w2 = rpool.tile([128, 1], f32, tag="w2")
                nc.vector.tensor_scalar(
                    w2[:], w1[:], -1.0, 1.0, op0=ALU.mult, op1=ALU.add
                )
                nc.vector.tensor_mul(i1[:], i1[:], w1[:])
                nc.vector.tensor_mul(i2[:], i2[:], w2[:])
                nc.vector.tensor_add(i1[:], i1[:], i2[:])
                nc.vector.tensor_mul(ce[:, j:j + 1], i1[:], valid[:, j:j + 1])

            # routed finalize + store overlaps the shared MLP
            for j in range(CJ):
                nc.vector.tensor_scalar_mul(y_r[:, j, :], y_r[:, j, :], ce[:, j:j + 1])
                nc.sync.dma_start(d_outr[:, j, :], y_r[:, j, :])

            def fin_sh(j):
                nc.sync.dma_start(d_outs[:, j, :], y_sh[:, j, :])

            mlp(xqb, d_wfcs, d_wpjs, y_sh, TQ, FSH, fin_sh, init=xres)

    if not nc.is_finalized():
        nc.finalize()
    return nc


def _tile128(a):
    """[N, M] (N % 128 == 0) -> [128, N//128, M] partition-tiled layout."""
    n, m = a.shape
    return np.ascontiguousarray(a.reshape(n // 128, 128, m).transpose(1, 0, 2))


def kernel(x, centroids, routing_bias, Wfc_r, Wproj_r, Wfc_sh, Wproj_sh):
    from concourse import bass_utils

    x = np.asarray(x, np.float32)
    centroids = np.asarray(centroids, np.float32)
    routing_bias = np.asarray(routing_bias, np.float32)
    Wfc_r = np.asarray(Wfc_r, np.float32)
    Wproj_r = np.asarray(Wproj_r, np.float32)
    Wfc_sh = np.asarray(Wfc_sh, np.float32)
    Wproj_sh = np.asarray(Wproj_sh, np.float32)

    xf = np.ascontiguousarray(x.reshape(TT, D))

    # host routing — data placement only (device recomputes combine weights)
    raw = xf @ centroids.T
    balanced = raw + routing_bias[None, :]
    top2 = np.argsort(-balanced, axis=-1, kind="stable")[:, :2]
    idx_lists = []
    for e in range(E):
        hit = (top2 == e).any(axis=1)
        idx_lists.append(np.nonzero(hit)[0].astype(np.int64))
    nmax = max(len(ix) for ix in idx_lists)
    C = max(256, ((nmax + 127) // 128) * 128)

    if (C, 1) not in _prog_cache:
        _prog_cache[(C, 1)] = _build(C, 1)
    nc = _prog_cache[(C, 1)]

    xT = np.ascontiguousarray(xf.T)
    ident = np.eye(128, dtype=np.float32)
    rbias128 = np.tile(routing_bias[None, :], (128, 1)).astype(np.float32)
    FH = F // 2  # shared expert F-half

    in_maps = []
    for c in range(E):
        ix = idx_lists[c]
        n = len(ix)
        pad = np.zeros(C, np.int64)
        pad[:n] = ix
        xg = xf[pad]  # [C, D]
        xgT_ = np.ascontiguousarray(xg.T)  # [D, C]
        validm = np.zeros((128, C // 128), np.float32)
        for t in range(C):
            if t < n:
                validm[t % 128, t // 128] = 1.0
        sel = np.zeros((128, E), np.float32)
        sel[:, c] = 1.0
        q = c // 2
        half = c % 2
        wfcsh = np.concatenate(
            [Wfc_sh[nn, half * FH:(half + 1) * FH, :] for nn in range(NS)], axis=0
        )  # [FSH, D]
        wpjsh = np.concatenate(
            [Wproj_sh[nn, :, half * FH:(half + 1) * FH] for nn in range(NS)], axis=1
        )  # [D, FSH]
        xqT_ = np.ascontiguousarray(xT[:, q * TQ:(q + 1) * TQ])  # [D, TQ]
        in_maps.append({
            "xt32": _tile128(xgT_),
            "xtb": _tile128(xgT_).astype(BF16),
            "xqb": _tile128(xqT_).astype(BF16),
            "x_res": _tile128(np.ascontiguousarray(xf[q * TQ:(q + 1) * TQ] * 0.5)),
            "centT": _tile128(np.ascontiguousarray(centroids.T)),
            "rbias": rbias128,
            "sel": sel,
            "valid": validm,
            "ident": ident,
            "wfcT": _tile128(np.ascontiguousarray(Wfc_r[c].T)).astype(BF16),
            "wprojT": _tile128(np.ascontiguousarray(Wproj_r[c].T)).astype(BF16),
            "wfcshT": _tile128(np.ascontiguousarray(wfcsh.T)).astype(BF16),
            "wprojshT": _tile128(np.ascontiguousarray(wpjsh.T)).astype(BF16),
        })

    globals()["_last_in_maps"] = in_maps
    globals()["_last_C"] = C
    res = bass_utils.run_bass_kernel_spmd(nc, in_maps, core_ids=list(range(E)))
    globals()["_last_results"] = res

    out = np.zeros((TT, D), np.float32)
    for c in range(E):
        r = res.results[c]
        q = c // 2
        # [128, nj, D] tiled -> [nj*128, D] token-major
        out_sh = np.asarray(r["out_sh"]).transpose(1, 0, 2).reshape(TQ, D)
        out[q * TQ:(q + 1) * TQ] += out_sh
        out_r = np.asarray(r["out_r"]).transpose(1, 0, 2).reshape(C, D)
        ix = idx_lists[c]
        out[ix] += out_r[:len(ix)]
    return out.reshape(B, S, D)


# revision 17
# speedup vs baseline: 121.6730x; 1.0046x over previous
"""MoE kernel for Trainium2 — 8-core expert-parallel + shared-expert 2D shard.

Strategy:
  - Host computes routing (replica of reference math, fp32) ONLY to decide
    data placement: which tokens go to which expert-core (top-2 dispatch).
    The combine weights used in the output math are recomputed ON DEVICE
    from raw inputs (centroid matmul in fp32 + sigmoid/top2/softmax).
  - Core e (e=0..7): routed expert e's MLP over its gathered tokens
    (capacity-padded to C, invalid rows masked to 0 on device), plus a
    (token-quarter x F-half) shard of the 2 shared experts.
  - MLP matmuls in bf16 (full PE rate, half the HBM traffic); routing in
    exact fp32. PSUM accumulation is always fp32.
  - All DRAM operands use a [128, ntile, free] "partition-tiled" layout so
    each weight chunk is a single >=1MB dma_start.
  - The whole per-iteration body can be wrapped in a hardware For_i loop
    (R iterations); iterations are idempotent, so the R>1 program computes
    the same output while letting a single NEFF dispatch execute the kernel
    R times back-to-back (used to measure true per-execution HW time under
    a high fixed dispatch latency).
  - Host unshard: scatter-add routed partials (unique indices per core),
    sum shared partials; residual x folded in on-device via x_res halves.
"""

import sys

sys.path.insert(0, "/opt/trn_rl_repo")

import numpy as np
import ml_dtypes

BF16 = ml_dtypes.bfloat16

D = 1024
F = 4096          # routed expert hidden
FSH = 4096        # shared shard hidden: 2 experts x (4096/2) F-half
E = 8
NS = 2
B, S = 2, 1024
TT = B * S        # 2048 tokens
TQ = TT // 4      # 512-token quarter per shared shard
FCH = 1024        # hidden-chunk streamed per iteration

_prog_cache = {}


def _mchunks(n):
    """Split n into moving-dim chunks of 512 + remainder. Chunks must start
    at multiples of 512 so no matmul output crosses a PSUM bank boundary
    (fp32 bank = 512 floats)."""
    out = [512] * (n // 512)
    if n % 512:
        out.append(n % 512)
    return out


def _build(C, R=1):
    from contextlib import ExitStack, nullcontext
    from concourse import bacc, bass, tile, mybir

    f32 = mybir.dt.float32
    bf16 = mybir.dt.bfloat16
    AF = mybir.ActivationFunctionType
    ALU = mybir.AluOpType
    AX = mybir.AxisListType
    ET = mybir.EngineType

    nc = bacc.Bacc("TRN2", target_bir_lowering=False, debug=False, num_devices=8)

    CJ = C // 128   # routed token tiles
    QJ = TQ // 128  # shared token tiles
    FJ = F // 128   # hidden tiles

    d_xt32 = nc.dram_tensor("xt32", [128, 8, C], f32, kind="ExternalInput").ap()
    d_xtb = nc.dram_tensor("xtb", [128, 8, C], bf16, kind="ExternalInput").ap()
    d_xqb = nc.dram_tensor("xqb", [128, 8, TQ], bf16, kind="ExternalInput").ap()
    d_xres = nc.dram_tensor("x_res", [128, QJ, D], f32, kind="ExternalInput").ap()
    d_centT = nc.dram_tensor("centT", [128, 8, E], f32, kind="ExternalInput").ap()
    d_rbias = nc.dram_tensor("rbias", [128, E], f32, kind="ExternalInput").ap()
    d_sel = nc.dram_tensor("sel", [128, E], f32, kind="ExternalInput").ap()
    d_valid = nc.dram_tensor("valid", [128, CJ], f32, kind="ExternalInput").ap()
    d_ident = nc.dram_tensor("ident", [128, 128], f32, kind="ExternalInput").ap()
    d_wfc = nc.dram_tensor("wfcT", [128, 8, F], bf16, kind="ExternalInput").ap()
    d_wpj = nc.dram_tensor("wprojT", [128, FJ, D], bf16, kind="ExternalInput").ap()
    d_wfcs = nc.dram_tensor("wfcshT", [128, 8, FSH], bf16, kind="ExternalInput").ap()
    d_wpjs = nc.dram_tensor("wprojshT", [128, FJ, D], bf16, kind="ExternalInput").ap()
    d_outr = nc.dram_tensor("out_r", [128, CJ, D], f32, kind="ExternalOutput").ap()
    d_outs = nc.dram_tensor("out_sh", [128, QJ, D], f32, kind="ExternalOutput").ap()

    with tile.TileContext(nc) as tc, ExitStack() as ctx:
        const = ctx.enter_context(tc.tile_pool(name="const", bufs=1))
        xpool = ctx.enter_context(tc.tile_pool(name="xpool", bufs=1))
        ypool = ctx.enter_context(tc.tile_pool(name="ypool", bufs=1))
        rpool = ctx.enter_context(tc.tile_pool(name="rpool", bufs=2))
        wpool = ctx.enter_context(tc.tile_pool(name="wpool", bufs=2))
        hpool = ctx.enter_context(tc.tile_pool(name="hpool", bufs=2))
        php = ctx.enter_context(tc.tile_pool(name="php", bufs=2, space="PSUM"))
        pyp = ctx.enter_context(tc.tile_pool(name="pyp", bufs=4, space="PSUM"))

        loop_cm = (
            tc.For_i(
                0, R, 1,
                hint_engines=(ET.PE, ET.DVE, ET.SP, ET.Activation),
                staggered_reset=True,
            )
            if R > 1
            else nullcontext()
        )
        with loop_cm:
            # DMA issue order is SP-FIFO: the routed-MLP operands go first so
            # PE work starts as early as possible; routing inputs follow and
            # routing overlaps the shared MLP.
            zbias = const.tile([128, 1], f32, tag="zbias")
            nc.vector.memset(zbias[:], 0.0)
            xgb = xpool.tile([128, 8, C], bf16, tag="xgb")

            y_r = ypool.tile([128, CJ, D], f32, tag="y_r")
            y_sh = ypool.tile([128, QJ, D], f32, tag="y_sh")
            ce = ypool.tile([128, CJ], f32, tag="ce")

            # ---- MLP (fc chunk ci overlaps proj of chunk ci-1) --------------
            def mlp(xsb, d_wfc_, d_wpj_, yacc, tcnt, fh, fin, x_dma=None,
                    init=None):
                njt = tcnt // 128
                nch = fh // FCH
                mt = FCH // 128

                def proj(ci, hs, wpj_t):
                    for j in range(njt):
                        pys = [
                            pyp.tile([128, 512], f32, tag="py", name=f"py{hi}")
                            for hi in range(D // 512)
                        ]
                        for mi in range(mt):
                            for hi, py in enumerate(pys):
                                nc.tensor.matmul(
                                    py[:],
                                    lhsT=hs[mi][:, j * 128:(j + 1) * 128],
                                    rhs=wpj_t[:, mi, hi * 512:(hi + 1) * 512],
                                    start=(mi == 0),
                                    stop=(mi == mt - 1),
                                )
                        for hi, py in enumerate(pys):
                            dst = yacc[:, j, hi * 512:(hi + 1) * 512]
                            if ci == 0:
                                if init is not None:
                                    nc.vector.tensor_add(
                                        dst, py[:], init[:, j, hi * 512:(hi + 1) * 512]
                                    )
                                else:
                                    nc.vector.tensor_copy(dst, py[:])
                            else:
                                nc.vector.tensor_add(dst, dst, py[:])
                        if ci == nch - 1 and fin is not None:
                            fin(j)

                prev = None
                for ci in range(nch):
                    wfc_t = wpool.tile([128, 8, FCH], bf16, tag="wfc")
                    if ci == 0:
                        # interleave the per-k x and weight loads so the first
                        # matmul's operands land first on the DMA rings
                        for di in range(8):
                            if x_dma is not None:
                                nc.sync.dma_start(xsb[:, di, :], x_dma[:, di, :])
                            nc.sync.dma_start(
                                wfc_t[:, di, :],
                                d_wfc_[:, di, ci * FCH:(ci + 1) * FCH],
                            )
                    else:
                        nc.sync.dma_start(
                            wfc_t[:], d_wfc_[:, :, ci * FCH:(ci + 1) * FCH]
                        )
                    wpj_t = wpool.tile([128, mt, D], bf16, tag="wpj")
                    nc.sync.dma_start(
                        wpj_t[:], d_wpj_[:, ci * mt:(ci + 1) * mt, :]
                    )
                    hs = []
                    for mi in range(mt):
                        ph = php.tile([128, tcnt], f32, tag="ph")
                        for k in range(8):
                            off = 0
                            for w in _mchunks(tcnt):
                                nc.tensor.matmul(
                                    ph[:, off:off + w],
                                    lhsT=wfc_t[:, k, mi * 128:(mi + 1) * 128],
                                    rhs=xsb[:, k, off:off + w],
                                    start=(k == 0),
                                    stop=(k == 7),
                                )
                                off += w
                        h = hpool.tile([128, tcnt], bf16, tag=f"h{mi}")
                        nc.scalar.activation(h[:], ph[:], AF.Gelu, bias=zbias[:])
                        hs.append(h)
                    if prev is not None:
                        proj(*prev)
                    prev = (ci, hs, wpj_t)
                proj(*prev)

            mlp(xgb, d_wfc, d_wpj, y_r, C, F, None, x_dma=d_xtb)

            # ---- routing inputs + shared operands ---------------------------
            xqb = xpool.tile([128, 8, TQ], bf16, tag="xqb")
            nc.sync.dma_start(xqb[:], d_xqb[:, :, :])
            xres = xpool.tile([128, QJ, D], f32, tag="xres")
            nc.sync.dma_start(xres[:], d_xres[:, :, :])
            xgT = xpool.tile([128, 8, C], f32, tag="xgT")
            nc.sync.dma_start(xgT[:], d_xt32[:, :, :])
            centT = const.tile([128, 8, E], f32, tag="cent")
            nc.sync.dma_start(centT[:], d_centT[:, :, :])
            rbias = const.tile([128, E], f32, tag="rbias")
            nc.sync.dma_start(rbias[:], d_rbias[:, :])
            sel = const.tile([128, E], f32, tag="sel")
            nc.sync.dma_start(sel[:], d_sel[:, :])
            valid = const.tile([128, CJ], f32, tag="valid")
            nc.sync.dma_start(valid[:], d_valid[:, :])
            ident = const.tile([128, 128], f32, tag="ident")
            nc.sync.dma_start(ident[:], d_ident[:, :])

            # ---- routing (fp32, device-side combine weights); overlaps the
            # shared MLP on PE/DVE ---------------------------------------------
            raw_ps = php.tile([E, C], f32, tag="ph")
            for k in range(8):
                off = 0
                for w in _mchunks(C):
                    nc.tensor.matmul(
                        raw_ps[:, off:off + w],
                        lhsT=centT[:, k, :],
                        rhs=xgT[:, k, off:off + w],
                        start=(k == 0),
                        stop=(k == 7),
                    )
                    off += w
            raw_sb = rpool.tile([E, C], f32, tag="rawsb")
            nc.vector.tensor_copy(raw_sb[:], raw_ps[:])

            for j in range(CJ):
                t_ps = pyp.tile([128, E], f32, tag="py")
                nc.tensor.transpose(
                    t_ps[:], raw_sb[:, j * 128:(j + 1) * 128], ident[0:E, 0:E]
                )
                raw_t = rpool.tile([128, E], f32, tag="rawt")
                nc.vector.tensor_copy(raw_t[:], t_ps[:])
                braw = rpool.tile([128, E], f32, tag="braw")
                nc.vector.tensor_add(braw[:], raw_t[:], rbias[:])
                # sigmoid via the tanh entry of the gelu table: keeping every
                # activation on one act-func set hoists the table load out of
                # the loop (a mid-iteration table switch stalls ACT ~1.3us)
                scores = rpool.tile([128, E], f32, tag="scores")
                nc.scalar.activation(
                    scores[:], raw_t[:], AF.Tanh, bias=zbias[:], scale=0.5
                )
                maxes = rpool.tile([128, 8], f32, tag="maxes")
                nc.vector.max(maxes[:], braw[:])
                mask1 = rpool.tile([128, E], f32, tag="mask1")
                nc.vector.tensor_tensor(
                    mask1[:], braw[:], maxes[:, 0:1].to_broadcast([128, E]), ALU.is_ge
                )
                mask2 = rpool.tile([128, E], f32, tag="mask2")
                nc.vector.tensor_tensor(
                    mask2[:], braw[:], maxes[:, 1:2].to_broadcast([128, E]), ALU.is_ge
                )
                nc.vector.tensor_sub(mask2[:], mask2[:], mask1[:])
                tmp = rpool.tile([128, E], f32, tag="tmp")
                s1 = rpool.tile([128, 1], f32, tag="s1")
                s2 = rpool.tile([128, 1], f32, tag="s2")
                nc.vector.tensor_mul(tmp[:], mask1[:], scores[:])
                nc.vector.tensor_reduce(s1[:], tmp[:], axis=AX.X, op=ALU.add)
                nc.vector.tensor_mul(tmp[:], mask2[:], scores[:])
                nc.vector.tensor_reduce(s2[:], tmp[:], axis=AX.X, op=ALU.add)
                i1 = rpool.tile([128, 1], f32, tag="i1")
                i2 = rpool.tile([128, 1], f32, tag="i2")
                nc.vector.tensor_mul(tmp[:], mask1[:], sel[:])
                nc.vector.tensor_reduce(i1[:], tmp[:], axis=AX.X, op=ALU.add)
                nc.vector.tensor_mul(tmp[:], mask2[:], sel[:])
                nc.vector.tensor_reduce(i2[:], tmp[:], axis=AX.X, op=ALU.add)
                # here scores = tanh(raw/2) = 2*sigmoid(raw)-1 and each mask
                # selects exactly one expert, so sig1-sig2 = (s1-s2)/2 and
                # w1 = sigmoid(sig1-sig2) = 0.5 + 0.5*tanh((s1-s2)/4)
                w1 = rpool.tile([128, 1], f32, tag="w1")
                nc.vector.tensor_sub(s1[:], s1[:], s2[:])
                nc.scalar.activation(
                    w1[:], s1[:], AF.Tanh, bias=zbias[:], scale=0.25
                )
                nc.vector.tensor_scalar(
                    w1[:], w1[:], 0.5, 0.5, op0=ALU.mult, op1=ALU.add
                )
                w2 = rpool.tile([128, 1], f32, tag="w2")
                nc.vector.tensor_scalar(
                    w2[:], w1[:], -1.0, 1.0, op0=ALU.mult, op1=ALU.add
                )
                nc.vector.tensor_mul(i1[:], i1[:], w1[:])
                nc.vector.tensor_mul(i2[:], i2[:], w2[:])
                nc.vector.tensor_add(i1[:], i1[:], i2[:])
                nc.vector.tensor_mul(ce[:, j:j + 1], i1[:], valid[:, j:j + 1])

            # routed finalize + store overlaps the shared MLP
            for j in range(CJ):
                nc.vector.tensor_scalar_mul(y_r[:, j, :], y_r[:, j, :], ce[:, j:j + 1])
                nc.sync.dma_start(d_outr[:, j, :], y_r[:, j, :])

            def fin_sh(j):
                nc.sync.dma_start(d_outs[:, j, :], y_sh[:, j, :])

            mlp(xqb, d_wfcs, d_wpjs, y_sh, TQ, FSH, fin_sh, init=xres)

    if not nc.is_finalized():
        nc.finalize()
    return nc


def _tile128(a):
    """[N, M] (N % 128 == 0) -> [128, N//128, M] partition-tiled layout."""
    n, m = a.shape
    return np.ascontiguousarray(a.reshape(n // 128, 128, m).transpose(1, 0, 2))


def kernel(x, centroids, routing_bias, Wfc_r, Wproj_r, Wfc_sh, Wproj_sh):
    from concourse import bass_utils

    x = np.asarray(x, np.float32)
    centroids = np.asarray(centroids, np.float32)
    routing_bias = np.asarray(routing_bias, np.float32)
    Wfc_r = np.asarray(Wfc_r, np.float32)
    Wproj_r = np.asarray(Wproj_r, np.float32)
    Wfc_sh = np.asarray(Wfc_sh, np.float32)
    Wproj_sh = np.asarray(Wproj_sh, np.float32)

    xf = np.ascontiguousarray(x.reshape(TT, D))

    # host routing — data placement only (device recomputes combine weights)
    raw = xf @ centroids.T
    balanced = raw + routing_bias[None, :]
    top2 = np.argsort(-balanced, axis=-1, kind="stable")[:, :2]
    idx_lists = []
    for e in range(E):
        hit = (top2 == e).any(axis=1)
        idx_lists.append(np.nonzero(hit)[0].astype(np.int64))
    nmax = max(len(ix) for ix in idx_lists)
    C = max(256, ((nmax + 127) // 128) * 128)

    if (C, 1) not in _prog_cache:
        _prog_cache[(C, 1)] = _build(C, 1)
    nc = _prog_cache[(C, 1)]

    xT = np.ascontiguousarray(xf.T)
    ident = np.eye(128, dtype=np.float32)
    rbias128 = np.tile(routing_bias[None, :], (128, 1)).astype(np.float32)
    FH = F // 2  # shared expert F-half

    in_maps = []
    for c in range(E):
        ix = idx_lists[c]
        n = len(ix)
        pad = np.zeros(C, np.int64)
        pad[:n] = ix
        xg = xf[pad]  # [C, D]
        xgT_ = np.ascontiguousarray(xg.T)  # [D, C]
        validm = np.zeros((128, C // 128), np.float32)
        for t in range(C):
            if t < n:
                validm[t % 128, t // 128] = 1.0
        sel = np.zeros((128, E), np.float32)
        sel[:, c] = 1.0
        q = c // 2
        half = c % 2
        wfcsh = np.concatenate(
            [Wfc_sh[nn, half * FH:(half + 1) * FH, :] for nn in range(NS)], axis=0
        )  # [FSH, D]
        wpjsh = np.concatenate(
            [Wproj_sh[nn, :, half * FH:(half + 1) * FH] for nn in range(NS)], axis=1
        )  # [D, FSH]
        xqT_ = np.ascontiguousarray(xT[:, q * TQ:(q + 1) * TQ])  # [D, TQ]
        in_maps.append({
            "xt32": _tile128(xgT_),
            "xtb": _tile128(xgT_).astype(BF16),
            "xqb": _tile128(xqT_).astype(BF16),
            "x_res": _tile128(np.ascontiguousarray(xf[q * TQ:(q + 1) * TQ] * 0.5)),
            "centT": _tile128(np.ascontiguousarray(centroids.T)),
            "rbias": rbias128,
            "sel": sel,
            "valid": validm,
            "ident": ident,
            "wfcT": _tile128(np.ascontiguousarray(Wfc_r[c].T)).astype(BF16),
            "wprojT": _tile128(np.ascontiguousarray(Wproj_r[c].T)).astype(BF16),
            "wfcshT": _tile128(np.ascontiguousarray(wfcsh.T)).astype(BF16),
            "wprojshT": _tile128(np.ascontiguousarray(wpjsh.T)).astype(BF16),
        })

    globals()["_last_in_maps"] = in_maps
    globals()["_last_C"] = C
    res = bass_utils.run_bass_kernel_spmd(nc, in_maps, core_ids=list(range(E)))
    globals()["_last_results"] = res

    out = np.zeros((TT, D), np.float32)
    for c in range(E):
        r = res.results[c]
        q = c // 2
        # [128, nj, D] tiled -> [nj*128, D] token-major
        out_sh = np.asarray(r["out_sh"]).transpose(1, 0, 2).reshape(TQ, D)
        out[q * TQ:(q + 1) * TQ] += out_sh
        out_r = np.asarray(r["out_r"]).transpose(1, 0, 2).reshape(C, D)
        ix = idx_lists[c]
        out[ix] += out_r[:len(ix)]
    return out.reshape(B, S, D)


# revision 19
# speedup vs baseline: 121.9381x; 1.0022x over previous
"""MoE kernel for Trainium2 — 8-core expert-parallel + shared-expert 2D shard.

Strategy:
  - Host computes routing (replica of reference math, fp32) ONLY to decide
    data placement: which tokens go to which expert-core (top-2 dispatch).
    The combine weights used in the output math are recomputed ON DEVICE
    from raw inputs (centroid matmul in fp32 + sigmoid/top2/softmax).
  - Core e (e=0..7): routed expert e's MLP over its gathered tokens
    (capacity-padded to C, invalid rows masked to 0 on device), plus a
    (token-quarter x F-half) shard of the 2 shared experts.
  - MLP matmuls in bf16 (full PE rate, half the HBM traffic); routing in
    exact fp32. PSUM accumulation is always fp32.
  - All DRAM operands use a [128, ntile, free] "partition-tiled" layout so
    each weight chunk is a single >=1MB dma_start.
  - The whole per-iteration body can be wrapped in a hardware For_i loop
    (R iterations); iterations are idempotent, so the R>1 program computes
    the same output while letting a single NEFF dispatch execute the kernel
    R times back-to-back (used to measure true per-execution HW time under
    a high fixed dispatch latency).
  - Host unshard: scatter-add routed partials (unique indices per core),
    sum shared partials; residual x folded in on-device via x_res halves.
"""

import sys

sys.path.insert(0, "/opt/trn_rl_repo")

import numpy as np
import ml_dtypes

BF16 = ml_dtypes.bfloat16

D = 1024
F = 4096          # routed expert hidden
FSH = 4096        # shared shard hidden: 2 experts x (4096/2) F-half
E = 8
NS = 2
B, S = 2, 1024
TT = B * S        # 2048 tokens
TQ = TT // 4      # 512-token quarter per shared shard
FCH = 1024        # hidden-chunk streamed per iteration

_prog_cache = {}


def _mchunks(n):
    """Split n into moving-dim chunks of 512 + remainder. Chunks must start
    at multiples of 512 so no matmul output crosses a PSUM bank boundary
    (fp32 bank = 512 floats)."""
    out = [512] * (n // 512)
    if n % 512:
        out.append(n % 512)
    return out


def _build(C, R=1):
    from contextlib import ExitStack, nullcontext
    from concourse import bacc, bass, tile, mybir

    f32 = mybir.dt.float32
    bf16 = mybir.dt.bfloat16
    AF = mybir.ActivationFunctionType
    ALU = mybir.AluOpType
    AX = mybir.AxisListType
    ET = mybir.EngineType

    nc = bacc.Bacc("TRN2", target_bir_lowering=False, debug=False, num_devices=8)

    CJ = C // 128   # routed token tiles
    QJ = TQ // 128  # shared token tiles
    FJ = F // 128   # hidden tiles

    d_xt32 = nc.dram_tensor("xt32", [128, 8, C], f32, kind="ExternalInput").ap()
    d_xtb = nc.dram_tensor("xtb", [128, 8, C], bf16, kind="ExternalInput").ap()
    d_xqb = nc.dram_tensor("xqb", [128, 8, TQ], bf16, kind="ExternalInput").ap()
    d_xres = nc.dram_tensor("x_res", [128, QJ, D], f32, kind="ExternalInput").ap()
    d_centT = nc.dram_tensor("centT", [128, 8, E], f32, kind="ExternalInput").ap()
    d_rbias = nc.dram_tensor("rbias", [128, E], f32, kind="ExternalInput").ap()
    d_sel = nc.dram_tensor("sel", [128, E], f32, kind="ExternalInput").ap()
    d_valid = nc.dram_tensor("valid", [128, CJ], f32, kind="ExternalInput").ap()
    d_ident = nc.dram_tensor("ident", [128, 128], f32, kind="ExternalInput").ap()
    d_wfc = nc.dram_tensor("wfcT", [128, 8, F], bf16, kind="ExternalInput").ap()
    d_wpj = nc.dram_tensor("wprojT", [128, FJ, D], bf16, kind="ExternalInput").ap()
    d_wfcs = nc.dram_tensor("wfcshT", [128, 8, FSH], bf16, kind="ExternalInput").ap()
    d_wpjs = nc.dram_tensor("wprojshT", [128, FJ, D], bf16, kind="ExternalInput").ap()
    d_outr = nc.dram_tensor("out_r", [128, CJ, D], f32, kind="ExternalOutput").ap()
    d_outs = nc.dram_tensor("out_sh", [128, QJ, D], f32, kind="ExternalOutput").ap()

    with tile.TileContext(nc) as tc, ExitStack() as ctx:
        const = ctx.enter_context(tc.tile_pool(name="const", bufs=1))
        xpool = ctx.enter_context(tc.tile_pool(name="xpool", bufs=1))
        ypool = ctx.enter_context(tc.tile_pool(name="ypool", bufs=1))
        rpool = ctx.enter_context(tc.tile_pool(name="rpool", bufs=2))
        wpool = ctx.enter_context(tc.tile_pool(name="wpool", bufs=2))
        hpool = ctx.enter_context(tc.tile_pool(name="hpool", bufs=2))
        php = ctx.enter_context(tc.tile_pool(name="php", bufs=2, space="PSUM"))
        pyp = ctx.enter_context(tc.tile_pool(name="pyp", bufs=4, space="PSUM"))

        loop_cm = (
            tc.For_i(
                0, R, 1,
                hint_engines=(ET.PE, ET.DVE, ET.SP, ET.Activation),
                staggered_reset=True,
            )
            if R > 1
            else nullcontext()
        )
        with loop_cm:
            # DMA issue order is SP-FIFO: the routed-MLP operands go first so
            # PE work starts as early as possible; routing inputs follow and
            # routing overlaps the shared MLP.
            zbias = const.tile([128, 1], f32, tag="zbias")
            nc.vector.memset(zbias[:], 0.0)
            xgb = xpool.tile([128, 8, C], bf16, tag="xgb")

            y_r = ypool.tile([128, CJ, D], f32, tag="y_r")
            y_sh = ypool.tile([128, QJ, D], f32, tag="y_sh")
            ce = ypool.tile([128, CJ], f32, tag="ce")

            # ---- MLP (fc chunk ci overlaps proj of chunk ci-1) --------------
            def mlp(xsb, d_wfc_, d_wpj_, yacc, tcnt, fh, fin, x_dma=None,
                    init=None):
                njt = tcnt // 128
                nch = fh // FCH
                mt = FCH // 128

                def proj(ci, hs, wpj_t):
                    for j in range(njt):
                        pys = [
                            pyp.tile([128, 512], f32, tag="py", name=f"py{hi}")
                            for hi in range(D // 512)
                        ]
                        for mi in range(mt):
                            for hi, py in enumerate(pys):
                                nc.tensor.matmul(
                                    py[:],
                                    lhsT=hs[mi][:, j * 128:(j + 1) * 128],
                                    rhs=wpj_t[:, mi, hi * 512:(hi + 1) * 512],
                                    start=(mi == 0),
                                    stop=(mi == mt - 1),
                                )
                        for hi, py in enumerate(pys):
                            dst = yacc[:, j, hi * 512:(hi + 1) * 512]
                            if ci == 0:
                                if init is not None:
                                    nc.vector.tensor_add(
                                        dst, py[:], init[:, j, hi * 512:(hi + 1) * 512]
                                    )
                                else:
                                    nc.vector.tensor_copy(dst, py[:])
                            else:
                                nc.vector.tensor_add(dst, dst, py[:])
                        if ci == nch - 1 and fin is not None:
                            fin(j)

                prev = None
                for ci in range(nch):
                    wfc_t = wpool.tile([128, 8, FCH], bf16, tag="wfc")
                    if ci == 0:
                        # interleave the per-k x and weight loads so the first
                        # matmul's operands land first on the DMA rings
                        for di in range(8):
                            if x_dma is not None:
                                nc.sync.dma_start(xsb[:, di, :], x_dma[:, di, :])
                            nc.sync.dma_start(
                                wfc_t[:, di, :],
                                d_wfc_[:, di, ci * FCH:(ci + 1) * FCH],
                            )
                    else:
                        nc.sync.dma_start(
                            wfc_t[:], d_wfc_[:, :, ci * FCH:(ci + 1) * FCH]
                        )
                    wpj_t = wpool.tile([128, mt, D], bf16, tag="wpj")
                    nc.sync.dma_start(
                        wpj_t[:], d_wpj_[:, ci * mt:(ci + 1) * mt, :]
                    )
                    hs = []
                    for mi in range(mt):
                        ph = php.tile([128, tcnt], f32, tag="ph")
                        for k in range(8):
                            off = 0
                            for w in _mchunks(tcnt):
                                nc.tensor.matmul(
                                    ph[:, off:off + w],
                                    lhsT=wfc_t[:, k, mi * 128:(mi + 1) * 128],
                                    rhs=xsb[:, k, off:off + w],
                                    start=(k == 0),
                                    stop=(k == 7),
                                )
                                off += w
                        h = hpool.tile([128, tcnt], bf16, tag=f"h{mi}")
                        nc.scalar.activation(h[:], ph[:], AF.Gelu, bias=zbias[:])
                        hs.append(h)
                    if prev is not None:
                        proj(*prev)
                    prev = (ci, hs, wpj_t)
                proj(*prev)

            mlp(xgb, d_wfc, d_wpj, y_r, C, F, None, x_dma=d_xtb)

            # ---- routing inputs + shared operands ---------------------------
            xqb = xpool.tile([128, 8, TQ], bf16, tag="xqb")
            nc.sync.dma_start(xqb[:], d_xqb[:, :, :])
            xres = xpool.tile([128, QJ, D], f32, tag="xres")
            nc.sync.dma_start(xres[:], d_xres[:, :, :])
            xgT = xpool.tile([128, 8, C], f32, tag="xgT")
            nc.sync.dma_start(xgT[:], d_xt32[:, :, :])
            centT = const.tile([128, 8, E], f32, tag="cent")
            nc.sync.dma_start(centT[:], d_centT[:, :, :])
            rbias = const.tile([128, E], f32, tag="rbias")
            nc.sync.dma_start(rbias[:], d_rbias[:, :])
            sel = const.tile([128, E], f32, tag="sel")
            nc.sync.dma_start(sel[:], d_sel[:, :])
            valid = const.tile([128, CJ], f32, tag="valid")
            nc.sync.dma_start(valid[:], d_valid[:, :])
            ident = const.tile([128, 128], f32, tag="ident")
            nc.sync.dma_start(ident[:], d_ident[:, :])

            # ---- routing (fp32, device-side combine weights); overlaps the
            # shared MLP on PE/DVE ---------------------------------------------
            raw_ps = php.tile([E, C], f32, tag="ph")
            for k in range(8):
                off = 0
                for w in _mchunks(C):
                    nc.tensor.matmul(
                        raw_ps[:, off:off + w],
                        lhsT=centT[:, k, :],
                        rhs=xgT[:, k, off:off + w],
                        start=(k == 0),
                        stop=(k == 7),
                    )
                    off += w
            raw_sb = rpool.tile([E, C], f32, tag="rawsb")
            nc.vector.tensor_copy(raw_sb[:], raw_ps[:])

            for j in range(CJ):
                t_ps = pyp.tile([128, E], f32, tag="py")
                nc.tensor.transpose(
                    t_ps[:], raw_sb[:, j * 128:(j + 1) * 128], ident[0:E, 0:E]
                )
                raw_t = rpool.tile([128, E], f32, tag="rawt")
                nc.vector.tensor_copy(raw_t[:], t_ps[:])
                braw = rpool.tile([128, E], f32, tag="braw")
                nc.vector.tensor_add(braw[:], raw_t[:], rbias[:])
                # sigmoid via the tanh entry of the gelu table: keeping every
                # activation on one act-func set hoists the table load out of
                # the loop (a mid-iteration table switch stalls ACT ~1.3us)
                scores = rpool.tile([128, E], f32, tag="scores")
                nc.scalar.activation(
                    scores[:], raw_t[:], AF.Tanh, bias=zbias[:], scale=0.5
                )
                maxes = rpool.tile([128, 8], f32, tag="maxes")
                nc.vector.max(maxes[:], braw[:])
                mask1 = rpool.tile([128, E], f32, tag="mask1")
                nc.vector.tensor_tensor(
                    mask1[:], braw[:], maxes[:, 0:1].to_broadcast([128, E]), ALU.is_ge
                )
                mask2 = rpool.tile([128, E], f32, tag="mask2")
                nc.vector.tensor_tensor(
                    mask2[:], braw[:], maxes[:, 1:2].to_broadcast([128, E]), ALU.is_ge
                )
                nc.vector.tensor_sub(mask2[:], mask2[:], mask1[:])
                tmp = rpool.tile([128, E], f32, tag="tmp")
                s1 = rpool.tile([128, 1], f32, tag="s1")
                s2 = rpool.tile([128, 1], f32, tag="s2")
                nc.vector.tensor_mul(tmp[:], mask1[:], scores[:])
                nc.vector.tensor_reduce(s1[:], tmp[:], axis=AX.X, op=ALU.add)
                nc.vector.tensor_mul(tmp[:], mask2[:], scores[:])
                nc.vector.tensor_reduce(s2[:], tmp[:], axis=AX.X, op=ALU.add)
                i1 = rpool.tile([128, 1], f32, tag="i1")
                i2 = rpool.tile([128, 1], f32, tag="i2")
                nc.vector.tensor_mul(tmp[:], mask1[:], sel[:])
                nc.vector.tensor_reduce(i1[:], tmp[:], axis=AX.X, op=ALU.add)
                nc.vector.tensor_mul(tmp[:], mask2[:], sel[:])
                nc.vector.tensor_reduce(i2[:], tmp[:], axis=AX.X, op=ALU.add)
                # here scores = tanh(raw/2) = 2*sigmoid(raw)-1 and each mask
                # selects exactly one expert, so sig1-sig2 = (s1-s2)/2 and
                # w1 = sigmoid(sig1-sig2) = 0.5 + 0.5*tanh((s1-s2)/4)
                w1 = rpool.tile([128, 1], f32, tag="w1")
                nc.vector.tensor_sub(s1[:], s1[:], s2[:])
                nc.scalar.activation(
                    w1[:], s1[:], AF.Tanh, bias=zbias[:], scale=0.25
                )
                nc.vector.tensor_scalar(
                    w1[:], w1[:], 0.5, 0.5, op0=ALU.mult, op1=ALU.add
                )
                w2 = rpool.tile([128, 1], f32, tag="w2")
                nc.vector.tensor_scalar(
                    w2[:], w1[:], -1.0, 1.0, op0=ALU.mult, op1=ALU.add
                )
                nc.vector.tensor_mul(i1[:], i1[:], w1[:])
                nc.vector.tensor_mul(i2[:], i2[:], w2[:])
                nc.vector.tensor_add(i1[:], i1[:], i2[:])
                nc.vector.tensor_mul(ce[:, j:j + 1], i1[:], valid[:, j:j + 1])

            # routed finalize + store overlaps the shared MLP
            for j in range(CJ):
                nc.vector.tensor_scalar_mul(y_r[:, j, :], y_r[:, j, :], ce[:, j:j + 1])
                nc.sync.dma_start(d_outr[:, j, :], y_r[:, j, :])

            def fin_sh(j):
                nc.sync.dma_start(d_outs[:, j, :], y_sh[:, j, :])

            mlp(xqb, d_wfcs, d_wpjs, y_sh, TQ, FSH, fin_sh, init=xres)

    if not nc.is_finalized():
        nc.finalize()
    return nc


def _tile128(a):
    """[N, M] (N % 128 == 0) -> [128, N//128, M] partition-tiled layout."""
    n, m = a.shape
    return np.ascontiguousarray(a.reshape(n // 128, 128, m).transpose(1, 0, 2))


def kernel(x, centroids, routing_bias, Wfc_r, Wproj_r, Wfc_sh, Wproj_sh):
    from concourse import bass_utils

    x = np.asarray(x, np.float32)
    centroids = np.asarray(centroids, np.float32)
    routing_bias = np.asarray(routing_bias, np.float32)
    Wfc_r = np.asarray(Wfc_r, np.float32)
    Wproj_r = np.asarray(Wproj_r, np.float32)
    Wfc_sh = np.asarray(Wfc_sh, np.float32)
    Wproj_sh = np.asarray(Wproj_sh, np.float32)

    xf = np.ascontiguousarray(x.reshape(TT, D))

    # host routing — data placement only (device recomputes combine weights)
    raw = xf @ centroids.T
    balanced = raw + routing_bias[None, :]
    top2 = np.argsort(-balanced, axis=-1, kind="stable")[:, :2]
    idx_lists = []
    for e in range(E):
        hit = (top2 == e).any(axis=1)
        idx_lists.append(np.nonzero(hit)[0].astype(np.int64))
    nmax = max(len(ix) for ix in idx_lists)
    C = max(256, ((nmax + 127) // 128) * 128)

    if (C, 1) not in _prog_cache:
        _prog_cache[(C, 1)] = _build(C, 1)
    nc = _prog_cache[(C, 1)]

    xT = np.ascontiguousarray(xf.T)
    ident = np.eye(128, dtype=np.float32)
    rbias128 = np.tile(routing_bias[None, :], (128, 1)).astype(np.float32)
    FH = F // 2  # shared expert F-half

    in_maps = []
    for c in range(E):
        ix = idx_lists[c]
        n = len(ix)
        pad = np.zeros(C, np.int64)
        pad[:n] = ix
        xg = xf[pad]  # [C, D]
        xgT_ = np.ascontiguousarray(xg.T)  # [D, C]
        validm = np.zeros((128, C // 128), np.float32)
        for t in range(C):
            if t < n:
                validm[t % 128, t // 128] = 1.0
        sel = np.zeros((128, E), np.float32)
        sel[:, c] = 1.0
        q = c // 2
        half = c % 2
        wfcsh = np.concatenate(
            [Wfc_sh[nn, half * FH:(half + 1) * FH, :] for nn in range(NS)], axis=0
        )  # [FSH, D]
        wpjsh = np.concatenate(
            [Wproj_sh[nn, :, half * FH:(half + 1) * FH] for nn in range(NS)], axis=1
        )  # [D, FSH]
        xqT_ = np.ascontiguousarray(xT[:, q * TQ:(q + 1) * TQ])  # [D, TQ]
        in_maps.append({
            "xt32": _tile128(xgT_),
            "xtb": _tile128(xgT_).astype(BF16),
            "xqb": _tile128(xqT_).astype(BF16),
            "x_res": _tile128(np.ascontiguousarray(xf[q * TQ:(q + 1) * TQ] * 0.5)),
            "centT": _tile128(np.ascontiguousarray(centroids.T)),
            "rbias": rbias128,
            "sel": sel,
            "valid": validm,
            "ident": ident,
            "wfcT": _tile128(np.ascontiguousarray(Wfc_r[c].T)).astype(BF16),
            "wprojT": _tile128(np.ascontiguousarray(Wproj_r[c].T)).astype(BF16),
            "wfcshT": _tile128(np.ascontiguousarray(wfcsh.T)).astype(BF16),
            "wprojshT": _tile128(np.ascontiguousarray(wpjsh.T)).astype(BF16),
        })

    globals()["_last_in_maps"] = in_maps
    globals()["_last_C"] = C
    res = bass_utils.run_bass_kernel_spmd(nc, in_maps, core_ids=list(range(E)))
    globals()["_last_results"] = res

    out = np.zeros((TT, D), np.float32)
    for c in range(E):
        r = res.results[c]
        q = c // 2
        # [128, nj, D] tiled -> [nj*128, D] token-major
        out_sh = np.asarray(r["out_sh"]).transpose(1, 0, 2).reshape(TQ, D)
        out[q * TQ:(q + 1) * TQ] += out_sh
        out_r = np.asarray(r["out_r"]).transpose(1, 0, 2).reshape(C, D)
        ix = idx_lists[c]
        out[ix] += out_r[:len(ix)]
    return out.reshape(B, S, D)
